# revision 1
# baseline (speedup 1.0000x reference)
"""Trainium2 Bass kernel for HNet attention (B=4, S=2048, H=768, 12 heads, RoPE, causal).

Sharding: 8 cores = 4 batches x 2 head-groups (6 heads each).
Wq/Wk/Wv split column-wise (head axis), Wo row-wise; host sums the two
partial o_proj outputs per batch (the "all-reduce" done at gather time).

Per-core dataflow (all matmuls in float32r = 1 cyc/row on the PE):
  xT [768,2048] (host-transposed) --PE--> Q,K,V natural [2048,384]
  RoPE on Q,K in natural layout (DVE/GPSIMD), PE-transpose -> QT,KT [384,2048]
  scoresT[k,q] = KT_tile.T @ QT  (per head, causal-trimmed strips)
  exp on ScalarE (no max subtraction; scores ~ N(0,1)), diag tiles masked
  PV: lhsT = [V_h | ones] [k,65] -> attn_outT rows 0:64 + softmax sums row 64
  sums -> (SBUF->SBUF DMA gather) -> reciprocal -> K=6 broadcast matmul -> scale
  o_proj: lhsT = attn_outT tiles, rhs = WoT -> out [2048,768] partial
"""

import os
import sys

import numpy as np

sys.path.insert(0, "/opt/trn_rl_repo")

from contextlib import ExitStack

import concourse.bacc as bacc
import concourse.tile as tile
from concourse import mybir
from concourse.bass_utils import run_bass_kernel_spmd

S = 2048
HID = 768
NH = 6            # heads per core
D = 64
F = NH * D        # 384 per-core feature slice
P = 128
SC = S // P       # 16
FC = HID // P     # 6
MC = F // P       # 3
QW = 512          # q strip width
NQ = S // QW      # 4
N_CORES = 8
ROPE_THETA = 10000.0

F32 = mybir.dt.float32
F32R = mybir.dt.float32r
AF = mybir.ActivationFunctionType


def _r(ap):
    """tiles are allocated as float32r already; keep as passthrough."""
    return ap


def _h3(ap):
    """[P, F] -> [P, NH, D] view."""
    return ap.rearrange("p (h d) -> p h d", h=NH)


def build_program():
    nc = bacc.Bacc("TRN2", target_bir_lowering=False, debug=False,
                   num_devices=N_CORES)

    xT_d = nc.dram_tensor("xT", [HID, S], F32R, kind="ExternalInput").ap()
    wqT_d = nc.dram_tensor("wqT", [HID, F], F32R, kind="ExternalInput").ap()
    wkT_d = nc.dram_tensor("wkT", [HID, F], F32R, kind="ExternalInput").ap()
    wvT_d = nc.dram_tensor("wvT", [HID, F], F32R, kind="ExternalInput").ap()
    woT_d = nc.dram_tensor("woT", [F, HID], F32R, kind="ExternalInput").ap()
    cos_d = nc.dram_tensor("cos6", [S, F], F32, kind="ExternalInput").ap()
    sin_d = nc.dram_tensor("sin6", [S, F], F32, kind="ExternalInput").ap()
    tri_d = nc.dram_tensor("tri", [P, P], F32R, kind="ExternalInput").ap()
    eye_d = nc.dram_tensor("eye", [P, P], F32R, kind="ExternalInput").ap()
    e_d = nc.dram_tensor("emat", [NH, F], F32R, kind="ExternalInput").ap()
    on_d = nc.dram_tensor("ones6", [P, NH], F32R, kind="ExternalInput").ap()
    out_d = nc.dram_tensor("out", [S, HID], F32, kind="ExternalOutput").ap()

    with tile.TileContext(nc) as tc, ExitStack() as ctx:
        const_pool = ctx.enter_context(tc.tile_pool(name="const", bufs=1))
        tri_sb = const_pool.tile([P, P], F32R, tag="tri")
        nc.sync.dma_start(tri_sb[:], tri_d[:])
        eye_sb = const_pool.tile([P, P], F32R, tag="eye")
        nc.sync.dma_start(eye_sb[:], eye_d[:])
        e_sb = const_pool.tile([NH, F], F32R, tag="emat")
        nc.sync.dma_start(e_sb[:], e_d[:])
        on_sb = const_pool.tile([P, NH], F32R, tag="ones6")
        nc.sync.dma_start(on_sb[:], on_d[:])

        # persistent per-phase tensors
        qkT_pool = ctx.enter_context(tc.tile_pool(name="qkT", bufs=1))
        kT = [qkT_pool.tile([P, S], F32R, tag=f"kT{m}", name=f"kT{m}") for m in range(MC)]
        v_pool = ctx.enter_context(tc.tile_pool(name="vp", bufs=1))
        v_sb = [v_pool.tile([P, NH * 65], F32R, tag=f"v{s}", name=f"v{s}") for s in range(SC)]
        ao_pool = ctx.enter_context(tc.tile_pool(name="ao", bufs=2))
        woT_pool = ctx.enter_context(tc.tile_pool(name="woT", bufs=1))
        woT = [woT_pool.tile([P, HID], F32R, tag=f"woT{m}", name=f"woT{m}") for m in range(MC)]
        for m in range(MC):
            nc.sync.dma_start(woT[m][:], woT_d[m * P:(m + 1) * P, :])

        # ---- single interleaved phase: proj-group(qc) then attention(qc) ----
        # PSUM banks (8): pq1 + pk1 + ring3 (pv/pt/sc/bp) + pvp1 + fin2
        with tc.tile_pool(name="xT", bufs=1) as xT_pool, \
             tc.tile_pool(name="wT", bufs=1) as wT_pool, \
             tc.tile_pool(name="ld", bufs=2) as ld_pool, \
             tc.tile_pool(name="rope", bufs=2) as rope_pool, \
             tc.tile_pool(name="ex", bufs=9) as ex_pool, \
             tc.tile_pool(name="stg", bufs=2) as stg_pool, \
             tc.tile_pool(name="sums", bufs=2) as sums_pool, \
             tc.tile_pool(name="ob", bufs=2) as ob_pool, \
             tc.tile_pool(name="ps_qk", bufs=1, space="PSUM") as ps_qk, \
             tc.tile_pool(name="ring", bufs=3, space="PSUM") as ring, \
             tc.tile_pool(name="ps_pv", bufs=2, space="PSUM") as ps_pv:

            xT = [xT_pool.tile([P, S], F32R, tag=f"xT{f}", name=f"xT{f}") for f in range(FC)]
            for f in range(FC):
                nc.sync.dma_start(xT[f][:], xT_d[f * P:(f + 1) * P, :])
            wqT = [wT_pool.tile([P, F], F32R, tag=f"wqT{f}", name=f"wqTs{f}") for f in range(FC)]
            wkT = [wT_pool.tile([P, F], F32R, tag=f"wkT{f}", name=f"wkTs{f}") for f in range(FC)]
            wvT = [wT_pool.tile([P, F], F32R, tag=f"wvT{f}", name=f"wvTs{f}") for f in range(FC)]
            for f in range(FC):
                fs = slice(f * P, (f + 1) * P)
                nc.sync.dma_start(wqT[f][:], wqT_d[fs, :])
                nc.sync.dma_start(wkT[f][:], wkT_d[fs, :])
                nc.sync.dma_start(wvT[f][:], wvT_d[fs, :])

            def emit_proj(s):
                sl = slice(s * P, (s + 1) * P)
                cs = ld_pool.tile([P, F], F32, tag="cos", name="cs")
                nc.sync.dma_start(cs[:], cos_d[sl, :])
                sn = ld_pool.tile([P, F], F32, tag="sin", name="sn")
                nc.sync.dma_start(sn[:], sin_d[sl, :])
                pq = ps_qk.tile([P, F], F32, tag="pq", name="pq")
                pk = ps_qk.tile([P, F], F32, tag="pk", name="pk")
                pv_ = ps_qk.tile([P, F], F32, tag="pv", name="pv")
                for f in range(FC):
                    st, sp = (f == 0), (f == FC - 1)
                    lhs = _r(xT[f][:, sl])
                    nc.tensor.matmul(pq[:], lhs, _r(wqT[f][:]), start=st, stop=sp)
                    nc.tensor.matmul(pk[:], lhs, _r(wkT[f][:]), start=st, stop=sp)
                    nc.tensor.matmul(pv_[:], lhs, _r(wvT[f][:]), start=st, stop=sp)
                return s, sl, cs, sn, pq, pk, pv_, None

            def emit_tail(state):
                s, sl, cs, sn, pq, pk, pv_, qTs = state
                qcol = (s % 4) * P
                dsts = {id(pq): (qTs, qcol), id(pk): (kT, sl)}
                # RoPE (natural layout): out = q*cos + rot_half(q)*sin_signed
                for pp in (pq, pk):
                    dstT, dcol = dsts[id(pp)]
                    t1 = rope_pool.tile([P, F], F32, tag="t1", name="t1")
                    nc.vector.tensor_mul(t1[:], pp[:], cs[:])
                    t2 = rope_pool.tile([P, F], F32, tag="t2", name="t2")
                    nc.vector.tensor_mul(_h3(t2)[:, :, 0:32],
                                         _h3(pp)[:, :, 32:64],
                                         _h3(sn)[:, :, 0:32])
                    nc.vector.tensor_mul(_h3(t2)[:, :, 32:64],
                                         _h3(pp)[:, :, 0:32],
                                         _h3(sn)[:, :, 32:64])
                    qr = rope_pool.tile([P, F], F32R, tag="qr", name="qr")
                    nc.gpsimd.tensor_add(qr[:], t1[:], t2[:])
                    for m in range(MC):
                        pt = ring.tile([P, QW], F32R, tag="ring", name="pt")
                        nc.tensor.transpose(_r(pt[:, 0:P]),
                                            _r(qr[:, m * P:(m + 1) * P]),
                                            _r(eye_sb[:]))
                        dc = dcol if pp is pk else slice(qcol, qcol + P)
                        nc.vector.tensor_copy(dstT[m][:, dc], pt[:, 0:P])
                # V with ones column per head: [V_h | 1] -> [P, NH*65]
                v3 = v_sb[s].rearrange("p (h e) -> p h e", h=NH)
                nc.vector.tensor_copy(v3[:, :, 0:64], _h3(pv_[:]))
                nc.vector.tensor_copy(v3[:, :, 64:65],
                                      on_sb.rearrange("p (h o) -> p h o", h=NH))

            # ---- helpers for interleaved emission ----
            def attn_pair(qc, m, qTs, aoT, sums):
                # heads 2m (rows 0:64) and 2m+1 (rows 64:128): their K=64
                # scores matmuls use disjoint PE row groups -> issue adjacent
                # so the PE reorder window runs them concurrently.
                q0 = qc * QW
                last = 4 * qc + 3
                pvps = [ps_pv.tile([65, QW], F32, tag="pvp", name="pvp")
                        for _ in range(2)]
                exs = []
                for kc in range(last + 1):
                    k0 = kc * P
                    qlo = max(q0, k0)
                    n = q0 + QW - qlo
                    pair = []
                    for par in range(2):
                        off = 64 * par
                        sp = ring.tile([P, QW], F32, tag="ring", name="sp")
                        nc.tensor.matmul(sp[:, 0:n],
                                         _r(kT[m][off:off + 64, k0:k0 + P]),
                                         _r(qTs[m][off:off + 64,
                                                   qlo - q0:qlo - q0 + n]),
                                         start=True, stop=True)
                        pair.append(sp)
                    for par in range(2):
                        ex = ex_pool.tile([P, QW], F32R, tag="ex", name="ex")
                        nc.scalar.activation(ex[:, 0:n], pair[par][:, 0:n],
                                             AF.Exp, scale=0.125)
                        if k0 >= q0:  # diagonal block: zero k > q
                            nc.gpsimd.tensor_mul(ex[:, 0:P], ex[:, 0:P],
                                                 tri_sb[:])
                        exs.append((par, kc, qlo, n, ex))
                for par, kc, qlo, n, ex in exs:
                    h = 2 * m + par
                    nc.tensor.matmul(pvps[par][:, qlo - q0:QW],
                                     _r(v_sb[kc][:, h * 65:h * 65 + 65]),
                                     _r(ex[:, 0:n]),
                                     start=(kc == 0), stop=(kc == last))
                for par in range(2):
                    h = 2 * m + par
                    off = 64 * par
                    nc.vector.tensor_copy(aoT[m][off:off + 64, :],
                                          pvps[par][0:64, :])
                    stg = stg_pool.tile([65, QW], F32, tag="stg", name="stg")
                    nc.vector.tensor_copy(stg[64:65, :], pvps[par][64:65, :])
                    nc.sync.dma_start(sums[h:h + 1, :], stg[64:65, :])

            def attn_post(qc, aoT, sums):
                q0 = qc * QW
                inv = sums_pool.tile([NH, QW], F32R, tag="inv", name="inv")
                with nc.allow_low_precision(reason="softmax sums"):
                    nc.vector.reciprocal(inv[:, :], sums[:, :])
                for m in range(MC):
                    bp = ring.tile([P, QW], F32, tag="ring", name="bp")
                    nc.tensor.matmul(bp[:], _r(e_sb[:, m * P:(m + 1) * P]),
                                     _r(inv[:, :]), start=True, stop=True)
                    nc.vector.tensor_mul(aoT[m][:, :], aoT[m][:, :], bp[:])
                for t in range(QW // P):
                    s0 = q0 + t * P
                    for half in range(2):
                        c0, c1 = half * F, half * F + F
                        fin = ring.tile([P, QW], F32, tag="ring", name="fin")
                        for m in range(MC):
                            nc.tensor.matmul(fin[:, 0:F],
                                             _r(aoT[m][:, s0 - q0:s0 - q0 + P]),
                                             _r(woT[m][:, c0:c1]),
                                             start=(m == 0), stop=(m == MC - 1))
                        ob = ob_pool.tile([P, F], F32, tag="ob", name="ob")
                        nc.vector.tensor_copy(ob[:], fin[:, 0:F])
                        nc.sync.dma_start(out_d[s0:s0 + P, c0:c1], ob[:])

            def alloc_strip(qc):
                qTs = [rope_pool.tile([P, QW], F32R, tag=f"qTs{m}", name=f"qTs{m}")
                       for m in range(MC)]
                aoT = [ao_pool.tile([P, QW], F32R, tag=f"aoTs{m}", name=f"aoTs{m}")
                       for m in range(MC)]
                sums = sums_pool.tile([NH, QW], F32, tag="sums", name="sums")
                return qTs, aoT, sums

            # ---- emission: group 0, then zip(attention qc, proj group qc+1) ----
            strips = {0: alloc_strip(0)}
            pending = None
            pending_post = None
            for s in range(4):
                state = emit_proj(s)[:-1] + (strips[0][0],)
                if pending is not None:
                    emit_tail(pending)
                pending = state

            for qc in range(NQ):
                qTs, aoT, sums = strips[qc]
                if qc + 1 < NQ:
                    strips[qc + 1] = alloc_strip(qc + 1)
                    nxt = list(range(4 * qc + 4, 4 * qc + 8))
                else:
                    nxt = []
                if not nxt and pending is not None:
                    emit_tail(pending)
                    pending = None
                for mp in range(MC + 1):
                    for s_i in nxt[mp:mp + 1]:
                        state = emit_proj(s_i)[:-1] + (strips[qc + 1][0],)
                        if pending is not None:
                            emit_tail(pending)
                        pending = state
                    if mp == 1 and pending_post is not None:
                        attn_post(*pending_post)
                        pending_post = None
                    if mp < MC:
                        attn_pair(qc, mp, qTs, aoT, sums)
                pending_post = (qc, aoT, sums)
            attn_post(*pending_post)
    nc.compile()
    return nc


def _rope_tables():
    inv_freq = 1.0 / (ROPE_THETA ** (np.arange(0, D, 2, dtype=np.float32) / D))
    t = np.arange(S, dtype=np.float32)
    freqs = np.outer(t, inv_freq)                       # [S, 32]
    emb = np.concatenate([freqs, freqs], axis=-1)       # [S, 64]
    cos = np.cos(emb).astype(np.float32)
    sin = np.sin(emb).astype(np.float32)
    sin_signed = sin.copy()
    sin_signed[:, 0:32] *= -1.0                         # fold rotate_half sign
    cos6 = np.tile(cos, (1, NH)).astype(np.float32)
    sin6 = np.tile(sin_signed, (1, NH)).astype(np.float32)
    return np.ascontiguousarray(cos6), np.ascontiguousarray(sin6)


_STATE = {}


def _get_program():
    if "nc" not in _STATE:
        _STATE["nc"] = build_program()
    return _STATE["nc"]


def _make_in_maps(hidden_states, Wq, Wk, Wv, Wo):
    hs = np.asarray(hidden_states, dtype=np.float32)
    Wq = np.asarray(Wq, dtype=np.float32)
    Wk = np.asarray(Wk, dtype=np.float32)
    Wv = np.asarray(Wv, dtype=np.float32)
    Wo = np.asarray(Wo, dtype=np.float32)

    cos6, sin6 = _rope_tables()
    tri = np.triu(np.ones((P, P), dtype=np.float32))        # j >= i keep
    eye = np.eye(P, dtype=np.float32)
    emat = np.repeat(np.eye(NH, dtype=np.float32), D, axis=1)  # [6, 384]

    in_maps = []
    for c in range(N_CORES):
        b, g = c // 2, c % 2
        cols = slice(g * F, (g + 1) * F)
        in_maps.append({
            "xT": np.ascontiguousarray(hs[b].T),                  # [768, S]
            "wqT": np.ascontiguousarray(Wq[cols, :].T),           # [768, 384]
            "wkT": np.ascontiguousarray(Wk[cols, :].T),
            "wvT": np.ascontiguousarray(Wv[cols, :].T),
            "woT": np.ascontiguousarray(Wo[:, cols].T),           # [384, 768]
            "cos6": cos6,
            "sin6": sin6,
            "tri": tri,
            "eye": eye,
            "emat": emat,
            "ones6": np.ones((P, NH), dtype=np.float32),
        })
    return in_maps


def run(hidden_states, Wq, Wk, Wv, Wo, trace=False, **trace_kw):
    nc = _get_program()
    in_maps = _make_in_maps(hidden_states, Wq, Wk, Wv, Wo)
    res = run_bass_kernel_spmd(nc, in_maps, core_ids=list(range(N_CORES)),
                               trace=trace, **trace_kw)
    B = 4
    out = np.empty((B, S, HID), dtype=np.float32)
    for b in range(B):
        out[b] = res.results[2 * b]["out"] + res.results[2 * b + 1]["out"]
    return out, res


def kernel(hidden_states, Wq, Wk, Wv, Wo):
    out, _ = run(hidden_states, Wq, Wk, Wv, Wo,
                 trace=bool(int(os.environ.get("KERNEL_TRACE", "0"))))
    return out



# revision 6
# speedup vs baseline: 1.0440x; 1.0440x over previous
"""Trainium2 Bass kernel for HNet attention (B=4, S=2048, H=768, 12 heads, RoPE, causal).

Sharding: 8 cores = 4 batches x 2 head-groups (6 heads each).
Wq/Wk/Wv split column-wise (head axis), Wo row-wise; host sums the two
partial o_proj outputs per batch (the "all-reduce" done at gather time).

Per-core dataflow (v2 — natural-layout PV, bf16 attention internals):
  xT [768,2048] (host-transposed, fp32r) --PE--> Q,K,V natural [2048,384]
  RoPE on Q,K in natural layout (DVE), PE-transpose (bf16) -> qT,kT [384,2048]
  scoresT[k,q] = kT.T @ qT per head pair (disjoint PE row groups), causal
    mask folded into the PE as an accumulate-matmul of a -1e9 triangle
  exp on ScalarE over merged head-pair psum tiles [128,1024] -> ex bf16
  PV natural: out[q, 65] += ex[k,q].T @ [V_h | 1]  (col 64 = softmax sums)
  normalize: reciprocal of sums col + per-head tensor_scalar -> ao_nat bf16
  PE-transpose ao_nat -> aoT; o_proj: fin = aoT.T @ woT (bf16) -> out fp32
"""

import os
import sys

import numpy as np

sys.path.insert(0, "/opt/trn_rl_repo")

from collections import deque
from contextlib import ExitStack

import ml_dtypes

import concourse.bacc as bacc
import concourse.tile as tile
from concourse import mybir
from concourse.bass_utils import run_bass_kernel_spmd

S = 2048
HID = 768
NH = 6            # heads per core
D = 64
F = NH * D        # 384 per-core feature slice
P = 128
SC = S // P       # 16 s-tiles
FC = HID // P     # 6 contraction chunks
MC = F // P       # 3 head-pair chunks
QW = 512          # q strip width
NQ = S // QW      # 4 strips
N_CORES = 8
ROPE_THETA = 10000.0
NEG = -1.0e9

F32 = mybir.dt.float32
F32R = mybir.dt.float32r
BF16 = mybir.dt.bfloat16
F16 = mybir.dt.float16
AF = mybir.ActivationFunctionType


def build_program():
    nc = bacc.Bacc("TRN2", target_bir_lowering=False, debug=False,
                   num_devices=N_CORES)

    xT_d = nc.dram_tensor("xT", [HID, S], F32R, kind="ExternalInput").ap()
    wqT_d = nc.dram_tensor("wqT", [HID, F], F32R, kind="ExternalInput").ap()
    wkT_d = nc.dram_tensor("wkT", [HID, F], F32R, kind="ExternalInput").ap()
    wvT_d = nc.dram_tensor("wvT", [HID, F], F32R, kind="ExternalInput").ap()
    woT_d = nc.dram_tensor("woT", [F, HID], BF16, kind="ExternalInput").ap()
    cos_d = nc.dram_tensor("cos6", [S, F], F16, kind="ExternalInput").ap()
    sin_d = nc.dram_tensor("sin6", [S, F], F16, kind="ExternalInput").ap()
    tri_d = nc.dram_tensor("trineg", [P, P], BF16, kind="ExternalInput").ap()
    eye_d = nc.dram_tensor("eye", [P, P], BF16, kind="ExternalInput").ap()
    out_d = nc.dram_tensor("out", [S, HID], F32, kind="ExternalOutput").ap()

    with tile.TileContext(nc) as tc, ExitStack() as ctx:
        const_pool = ctx.enter_context(tc.tile_pool(name="const", bufs=1))
        eye_sb = const_pool.tile([P, P], BF16, tag="eye")
        nc.sync.dma_start(eye_sb[:], eye_d[:])
        tri_sb = const_pool.tile([P, P], BF16, tag="tri")
        nc.sync.dma_start(tri_sb[:], tri_d[:])

        # ---- persistent SBUF ----
        xw_pool = ctx.enter_context(tc.tile_pool(name="xw", bufs=1))
        xT = [xw_pool.tile([P, S], F32R, tag=f"xT{f}", name=f"xT{f}")
              for f in range(FC)]
        wqT = [xw_pool.tile([P, F], F32R, tag=f"wqT{f}", name=f"wqT{f}")
               for f in range(FC)]
        wkT = [xw_pool.tile([P, F], F32R, tag=f"wkT{f}", name=f"wkT{f}")
               for f in range(FC)]
        wvT = [xw_pool.tile([P, F], F32R, tag=f"wvT{f}", name=f"wvT{f}")
               for f in range(FC)]
        woT = [xw_pool.tile([P, HID], BF16, tag=f"woT{m}", name=f"woT{m}")
               for m in range(MC)]
        # DMA emission order feeds the prologue: per f-chunk the weights,
        # then the first strip-column of xT, so proj(s0..3) can trickle
        # behind the DMA queue instead of waiting for all 9.5 MB.
        for f in range(FC):
            fs = slice(f * P, (f + 1) * P)
            nc.sync.dma_start(wqT[f][:], wqT_d[fs, :])
            nc.sync.dma_start(wkT[f][:], wkT_d[fs, :])
            nc.sync.dma_start(wvT[f][:], wvT_d[fs, :])
            nc.sync.dma_start(xT[f][:, 0:QW], xT_d[fs, 0:QW])
        for m in range(MC):
            nc.sync.dma_start(woT[m][:], woT_d[m * P:(m + 1) * P, :])
        for c in range(1, NQ):
            cl = slice(c * QW, (c + 1) * QW)
            for f in range(FC):
                nc.sync.dma_start(xT[f][:, cl], xT_d[f * P:(f + 1) * P, cl])

        kT_pool = ctx.enter_context(tc.tile_pool(name="kTp", bufs=1))
        kT = [kT_pool.tile([P, S], BF16, tag=f"kT{m}", name=f"kT{m}")
              for m in range(MC)]
        v_pool = ctx.enter_context(tc.tile_pool(name="vp", bufs=1))
        vo = [v_pool.tile([P, NH * 65], BF16, tag=f"v{s}", name=f"v{s}")
              for s in range(SC)]
        for s in range(SC):
            v3 = vo[s].rearrange("p (h e) -> p h e", h=NH)
            nc.gpsimd.memset(v3[:, :, 64], 1.0)

        with tc.tile_pool(name="ld", bufs=2) as ld_pool, \
             tc.tile_pool(name="rp", bufs=2) as rp_pool, \
             tc.tile_pool(name="qr", bufs=4) as qr_pool, \
             tc.tile_pool(name="qTs", bufs=4) as qTs_pool, \
             tc.tile_pool(name="ao", bufs=3) as ao_pool, \
             tc.tile_pool(name="ex", bufs=8) as ex_pool, \
             tc.tile_pool(name="an", bufs=2) as an_pool, \
             tc.tile_pool(name="iv", bufs=2) as iv_pool, \
             tc.tile_pool(name="ob", bufs=3) as ob_pool, \
             tc.tile_pool(name="mx", bufs=2, space="PSUM") as mx, \
             tc.tile_pool(name="sc", bufs=1, space="PSUM") as scp, \
             tc.tile_pool(name="po", bufs=4, space="PSUM") as pop:

            qTs = {}   # strip qc -> [3 tiles [P, QW] bf16]
            aoT = {}   # strip qc -> [3 tiles [P, QW] bf16]
            exs = {}   # (kc) -> ex pair tile for current strip
            pvb = {}   # live projB state per s

            def rope(pp, cs, sn):
                """psum natural QKV chunk [P, F] -> rotated bf16 sbuf tile."""
                p3 = pp.rearrange("p (h d) -> p h d", h=NH)
                c3 = cs.rearrange("p (h d) -> p h d", h=NH)
                s3 = sn.rearrange("p (h d) -> p h d", h=NH)
                t1 = rp_pool.tile([P, F], F32, tag="t1", name="t1")
                nc.vector.tensor_mul(t1[:], pp[:], cs[:])
                t2 = rp_pool.tile([P, F], F32, tag="t2", name="t2")
                t23 = t2.rearrange("p (h d) -> p h d", h=NH)
                nc.vector.tensor_mul(t23[:, :, 0:32], p3[:, :, 32:64],
                                     s3[:, :, 0:32])
                nc.vector.tensor_mul(t23[:, :, 32:64], p3[:, :, 0:32],
                                     s3[:, :, 32:64])
                qr = qr_pool.tile([P, F], BF16, tag="qr", name="qr")
                nc.vector.tensor_add(qr[:], t1[:], t2[:])
                return qr

            def emit_projA(s):
                sl = slice(s * P, (s + 1) * P)
                qc = s // NQ
                if s % 4 == 0:
                    qTs[qc] = [qTs_pool.tile([P, QW], BF16, tag=f"qTs{m}",
                                             name=f"qTs{m}") for m in range(MC)]
                cs = ld_pool.tile([P, F], F16, tag="cos", name="cs")
                nc.sync.dma_start(cs[:], cos_d[sl, :])
                sn = ld_pool.tile([P, F], F16, tag="sin", name="sn")
                nc.sync.dma_start(sn[:], sin_d[sl, :])
                pq = mx.tile([P, F], F32, tag="mx", name="pq")
                for f in range(FC):
                    nc.tensor.matmul(pq[:], xT[f][:, sl], wqT[f][:],
                                     start=(f == 0), stop=(f == FC - 1))
                pk = mx.tile([P, F], F32, tag="mx", name="pk")
                for f in range(FC):
                    nc.tensor.matmul(pk[:], xT[f][:, sl], wkT[f][:],
                                     start=(f == 0), stop=(f == FC - 1))
                qr = rope(pq, cs, sn)
                return (s, sl, cs, sn, pk, qr)

            def emit_projB(state):
                s, sl, cs, sn, pk, qr = state
                qc, scol = s // NQ, (s % 4) * P
                pv_ = mx.tile([P, F], F32, tag="mx", name="pv")
                for f in range(FC):
                    nc.tensor.matmul(pv_[:], xT[f][:, sl], wvT[f][:],
                                     start=(f == 0), stop=(f == FC - 1))
                kr = rope(pk, cs, sn)
                for m in range(MC):
                    tp = mx.tile([P, P], BF16, tag="mx", name="tpq")
                    nc.tensor.transpose(tp[:], qr[:, m * P:(m + 1) * P],
                                        eye_sb[:])
                    nc.gpsimd.tensor_copy(qTs[qc][m][:, scol:scol + P], tp[:])
                for m in range(MC):
                    tp = mx.tile([P, P], BF16, tag="mx", name="tpk")
                    nc.tensor.transpose(tp[:], kr[:, m * P:(m + 1) * P],
                                        eye_sb[:])
                    nc.gpsimd.tensor_copy(kT[m][:, sl], tp[:])
                v3 = vo[s].rearrange("p (h e) -> p h e", h=NH)
                p3 = pv_.rearrange("p (h d) -> p h d", h=NH)
                nc.gpsimd.tensor_copy(v3[:, :, 0:64], p3[:])

            def emit_scores(qc, kc, m):
                """head-pair m scores for block kc of strip qc + exp."""
                q0, k0 = qc * QW, kc * P
                off = max(0, k0 - q0)
                sp = scp.tile([P, 2 * QW], F32, tag="sc", name="sp")
                for par in range(2):
                    b = par * QW
                    d0 = 64 * par
                    lhsT = kT[m][d0:d0 + 64, k0:k0 + P]
                    rhs = qTs[qc][m]
                    if k0 >= q0:  # diagonal block: fold mask into PE
                        nc.tensor.matmul(sp[:, b + off:b + off + P],
                                         lhsT, rhs[d0:d0 + 64, off:off + P],
                                         start=True, stop=False)
                        nc.tensor.matmul(sp[:, b + off:b + off + P],
                                         eye_sb[:], tri_sb[:],
                                         start=False, stop=True)
                        if off + P < QW:
                            nc.tensor.matmul(sp[:, b + off + P:b + QW],
                                             lhsT, rhs[d0:d0 + 64, off + P:QW],
                                             start=True, stop=True)
                    else:
                        nc.tensor.matmul(sp[:, b:b + QW],
                                         lhsT, rhs[d0:d0 + 64, :],
                                         start=True, stop=True)
                ex = ex_pool.tile([P, 2 * QW], BF16, tag="ex", name="ex")
                sp3 = sp.rearrange("p (r c) -> p r c", r=2)
                ex3 = ex.rearrange("p (r c) -> p r c", r=2)
                nc.scalar.activation(ex3[:, :, off:QW], sp3[:, :, off:QW],
                                     AF.Exp, scale=0.125)
                exs[(kc, m)] = ex

            def emit_pv(qc, kc, m, po4):
                """consume ex[(kc, m)]: accumulate into every live q-tile."""
                ex = exs.pop((kc, m))
                for qt in range(max(0, kc - NQ * qc), NQ):
                    t = NQ * qc + qt
                    for par in range(2):
                        h = 2 * m + par
                        lhsT = ex[:, par * QW + qt * P:par * QW + qt * P + P]
                        nc.tensor.matmul(po4[qt][:, h * 65:h * 65 + 65],
                                         lhsT, vo[kc][:, h * 65:h * 65 + 65],
                                         start=(kc == 0), stop=(kc == t))

            def emit_norm(qc, qt, po):
                if qt == 0:
                    aoT[qc] = [ao_pool.tile([P, QW], BF16, tag=f"aoT{m}",
                                            name=f"aoT{m}") for m in range(MC)]
                po3 = po.rearrange("p (h e) -> p h e", h=NH)
                inv = iv_pool.tile([P, NH], F32, tag="inv", name="inv")
                with nc.allow_low_precision(reason="softmax sums"):
                    nc.vector.reciprocal(inv[:], po3[:, :, 64])
                an = an_pool.tile([P, F], BF16, tag="an", name="an")
                for h in range(NH):
                    nc.vector.tensor_scalar_mul(an[:, h * D:(h + 1) * D],
                                                po3[:, h, 0:D],
                                                inv[:, h:h + 1])
                for m in range(MC):
                    tp = mx.tile([P, P], BF16, tag="mx", name="tpa")
                    nc.tensor.transpose(tp[:], an[:, m * P:(m + 1) * P],
                                        eye_sb[:])
                    nc.vector.tensor_copy(aoT[qc][m][:, qt * P:(qt + 1) * P],
                                          tp[:])

            def emit_oproj(qc, qt):
                s0 = (NQ * qc + qt) * P
                for half in range(2):
                    c0 = half * F
                    fin = mx.tile([P, F], F32, tag="mx", name="fin")
                    for m in range(MC):
                        nc.tensor.matmul(fin[:],
                                         aoT[qc][m][:, qt * P:(qt + 1) * P],
                                         woT[m][:, c0:c0 + F],
                                         start=(m == 0), stop=(m == MC - 1))
                    ob = ob_pool.tile([P, F], F32, tag="ob", name="ob")
                    nc.gpsimd.tensor_copy(ob[:], fin[:])
                    nc.sync.dma_start(out_d[s0:s0 + P, c0:c0 + F], ob[:])

            # ---- emission schedule ----
            # fills: PE work units consumed 1-2 per scores block to keep the
            # PE busy during the Act(exp)-bound late strips. Proj A/B pairs
            # go in dependency order; o_proj units are appended as strips
            # complete and drain after the proj units run out.
            fills = deque()
            for s in range(4, SC):
                fills.append(("A", s))
                fills.append(("B", s))

            def pop_fill():
                if not fills:
                    return
                kind, a = fills.popleft()
                if kind == "A":
                    pvb[a] = emit_projA(a)
                elif kind == "B":
                    emit_projB(pvb.pop(a))
                else:
                    emit_oproj(*a)

            for s in range(4):
                st = emit_projA(s)
                emit_projB(st)

            for qc in range(NQ):
                exs.clear()
                last = NQ * qc + 3
                po4 = [pop.tile([P, NH * 65], F32, tag="po", name="po")
                       for _ in range(NQ)]
                for kc in range(last + 1):
                    pop_fill()
                    for m in range(MC):
                        emit_scores(qc, kc, m)
                        if kc > 0:
                            emit_pv(qc, kc - 1, m, po4)
                    qt_done = kc - 1 - NQ * qc
                    if 0 <= qt_done < NQ:
                        emit_norm(qc, qt_done, po4[qt_done])
                        fills.append(("O", (qc, qt_done)))
                for m in range(MC):
                    emit_pv(qc, last, m, po4)
                emit_norm(qc, NQ - 1, po4[NQ - 1])
                fills.append(("O", (qc, NQ - 1)))
            while fills:
                pop_fill()
    nc.compile()
    return nc


def _rope_tables():
    inv_freq = 1.0 / (ROPE_THETA ** (np.arange(0, D, 2, dtype=np.float32) / D))
    t = np.arange(S, dtype=np.float32)
    freqs = np.outer(t, inv_freq)                       # [S, 32]
    emb = np.concatenate([freqs, freqs], axis=-1)       # [S, 64]
    cos = np.cos(emb).astype(np.float32)
    sin = np.sin(emb).astype(np.float32)
    sin_signed = sin.copy()
    sin_signed[:, 0:32] *= -1.0                         # fold rotate_half sign
    cos6 = np.tile(cos, (1, NH)).astype(np.float16)
    sin6 = np.tile(sin_signed, (1, NH)).astype(np.float16)
    return np.ascontiguousarray(cos6), np.ascontiguousarray(sin6)


_STATE = {}


def _get_program():
    if "nc" not in _STATE:
        _STATE["nc"] = build_program()
    return _STATE["nc"]


def _make_in_maps(hidden_states, Wq, Wk, Wv, Wo):
    hs = np.asarray(hidden_states, dtype=np.float32)
    Wq = np.asarray(Wq, dtype=np.float32)
    Wk = np.asarray(Wk, dtype=np.float32)
    Wv = np.asarray(Wv, dtype=np.float32)
    Wo = np.asarray(Wo, dtype=np.float32)

    cos6, sin6 = _rope_tables()
    trineg = (NEG * np.tril(np.ones((P, P), dtype=np.float32), -1)
              ).astype(ml_dtypes.bfloat16)
    eye = np.eye(P, dtype=np.float32).astype(ml_dtypes.bfloat16)

    in_maps = []
    for c in range(N_CORES):
        b, g = c // 2, c % 2
        cols = slice(g * F, (g + 1) * F)
        in_maps.append({
            "xT": np.ascontiguousarray(hs[b].T),                  # [768, S]
            "wqT": np.ascontiguousarray(Wq[cols, :].T),           # [768, 384]
            "wkT": np.ascontiguousarray(Wk[cols, :].T),
            "wvT": np.ascontiguousarray(Wv[cols, :].T),
            "woT": np.ascontiguousarray(Wo[:, cols].T).astype(ml_dtypes.bfloat16),
            "cos6": cos6,
            "sin6": sin6,
            "trineg": trineg,
            "eye": eye,
        })
    return in_maps


def run(hidden_states, Wq, Wk, Wv, Wo, trace=False, **trace_kw):
    nc = _get_program()
    in_maps = _make_in_maps(hidden_states, Wq, Wk, Wv, Wo)
    res = run_bass_kernel_spmd(nc, in_maps, core_ids=list(range(N_CORES)),
                               trace=trace, **trace_kw)
    B = 4
    out = np.empty((B, S, HID), dtype=np.float32)
    for b in range(B):
        out[b] = res.results[2 * b]["out"] + res.results[2 * b + 1]["out"]
    return out, res


def kernel(hidden_states, Wq, Wk, Wv, Wo):
    out, _ = run(hidden_states, Wq, Wk, Wv, Wo,
                 trace=bool(int(os.environ.get("KERNEL_TRACE", "0"))))
    return out


# revision 10
# speedup vs baseline: 1.2480x; 1.1954x over previous
"""Trainium2 Bass kernel for HNet attention (B=4, S=2048, H=768, 12 heads, RoPE, causal).

Sharding: 8 cores = 4 batches x 2 head-groups (6 heads each).
Wq/Wk/Wv split column-wise (head axis), Wo row-wise; host sums the two
partial o_proj outputs per batch (the "all-reduce" done at gather time).

Per-core dataflow (v2 — natural-layout PV, bf16 attention internals):
  xT [768,2048] (host-transposed, fp32r) --PE--> Q,K,V natural [2048,384]
  RoPE on Q,K in natural layout (DVE), PE-transpose (bf16) -> qT,kT [384,2048]
  scoresT[k,q] = kT.T @ qT per head pair (disjoint PE row groups), causal
    mask folded into the PE as an accumulate-matmul of a -1e9 triangle
  exp on ScalarE over merged head-pair psum tiles [128,1024] -> ex bf16
  PV natural: out[q, 65] += ex[k,q].T @ [V_h | 1]  (col 64 = softmax sums)
  normalize: reciprocal of sums col + per-head tensor_scalar -> ao_nat bf16
  PE-transpose ao_nat -> aoT; o_proj: fin = aoT.T @ woT (bf16) -> out fp32
"""

import os
import sys

import numpy as np

sys.path.insert(0, "/opt/trn_rl_repo")

from collections import deque
from contextlib import ExitStack

import ml_dtypes

import concourse.bacc as bacc
import concourse.tile as tile
from concourse import mybir
from concourse.bass_utils import run_bass_kernel_spmd

S = 2048
HID = 768
NH = 6            # heads per core
D = 64
F = NH * D        # 384 per-core feature slice
P = 128
SC = S // P       # 16 s-tiles
FC = HID // P     # 6 contraction chunks
MC = F // P       # 3 head-pair chunks
QW = 512          # q strip width
NQ = S // QW      # 4 strips
N_CORES = 8
ROPE_THETA = 10000.0
NEG = -30000.0

F32 = mybir.dt.float32
F32R = mybir.dt.float32r
BF16 = mybir.dt.bfloat16
F16 = mybir.dt.float16
AF = mybir.ActivationFunctionType


def build_program():
    nc = bacc.Bacc("TRN2", target_bir_lowering=False, debug=False,
                   num_devices=N_CORES)

    xT_d = nc.dram_tensor("xT", [HID, S], F16, kind="ExternalInput").ap()
    wqT_d = nc.dram_tensor("wqT", [HID, F], F16, kind="ExternalInput").ap()
    wkT_d = nc.dram_tensor("wkT", [HID, F], F16, kind="ExternalInput").ap()
    wvT_d = nc.dram_tensor("wvT", [HID, F], F16, kind="ExternalInput").ap()
    woT_d = nc.dram_tensor("woT", [F, HID], F16, kind="ExternalInput").ap()
    cos_d = nc.dram_tensor("cos6", [S, F], F16, kind="ExternalInput").ap()
    sin_d = nc.dram_tensor("sin6", [S, F], F16, kind="ExternalInput").ap()
    tri_d = nc.dram_tensor("trineg", [P, P], F16, kind="ExternalInput").ap()
    eye_d = nc.dram_tensor("eye", [P, P], F16, kind="ExternalInput").ap()
    out_d = nc.dram_tensor("out", [S, HID], F32, kind="ExternalOutput").ap()

    with tile.TileContext(nc) as tc, ExitStack() as ctx:
        const_pool = ctx.enter_context(tc.tile_pool(name="const", bufs=1))
        eye_sb = const_pool.tile([P, P], F16, tag="eye")
        nc.sync.dma_start(eye_sb[:], eye_d[:])
        tri_sb = const_pool.tile([P, P], F16, tag="tri")
        nc.sync.dma_start(tri_sb[:], tri_d[:])

        # ---- persistent SBUF ----
        xw_pool = ctx.enter_context(tc.tile_pool(name="xw", bufs=1))
        xT = [xw_pool.tile([P, S], F16, tag=f"xT{f}", name=f"xT{f}")
              for f in range(FC)]
        wqT = [xw_pool.tile([P, F], F16, tag=f"wqT{f}", name=f"wqT{f}")
               for f in range(FC)]
        wkT = [xw_pool.tile([P, F], F16, tag=f"wkT{f}", name=f"wkT{f}")
               for f in range(FC)]
        wvT = [xw_pool.tile([P, F], F16, tag=f"wvT{f}", name=f"wvT{f}")
               for f in range(FC)]
        woT = [xw_pool.tile([P, HID], F16, tag=f"woT{m}", name=f"woT{m}")
               for m in range(MC)]
        # DMA emission order feeds the prologue: per f-chunk the weights,
        # then the first strip-column of xT, so proj(s0..3) can trickle
        # behind the DMA queue instead of waiting for all 9.5 MB.
        for f in range(FC):
            fs = slice(f * P, (f + 1) * P)
            nc.sync.dma_start(wqT[f][:], wqT_d[fs, :])
            nc.sync.dma_start(wkT[f][:], wkT_d[fs, :])
            nc.sync.dma_start(wvT[f][:], wvT_d[fs, :])
            nc.sync.dma_start(xT[f][:, 0:QW], xT_d[fs, 0:QW])
        for m in range(MC):
            nc.sync.dma_start(woT[m][:], woT_d[m * P:(m + 1) * P, :])

        kT_pool = ctx.enter_context(tc.tile_pool(name="kTp", bufs=1))
        kT = [kT_pool.tile([P, S], F16, tag=f"kT{m}", name=f"kT{m}")
              for m in range(MC)]
        v_pool = ctx.enter_context(tc.tile_pool(name="vp", bufs=1))
        vo = [v_pool.tile([P, NH * 65], F16, tag=f"v{s}", name=f"v{s}")
              for s in range(SC)]
        for s in range(SC):
            v3 = vo[s].rearrange("p (h e) -> p h e", h=NH)
            nc.gpsimd.memset(v3[:, :, 64], 1.0)

        with tc.tile_pool(name="ld", bufs=4) as ld_pool, \
             tc.tile_pool(name="rp", bufs=2) as rp_pool, \
             tc.tile_pool(name="qr", bufs=4) as qr_pool, \
             tc.tile_pool(name="qTs", bufs=4) as qTs_pool, \
             tc.tile_pool(name="ao", bufs=3) as ao_pool, \
             tc.tile_pool(name="ex", bufs=12) as ex_pool, \
             tc.tile_pool(name="an", bufs=2) as an_pool, \
             tc.tile_pool(name="iv", bufs=2) as iv_pool, \
             tc.tile_pool(name="ob", bufs=3) as ob_pool, \
             tc.tile_pool(name="mx", bufs=2, space="PSUM") as mx, \
             tc.tile_pool(name="sc", bufs=2, space="PSUM") as scp, \
             tc.tile_pool(name="po", bufs=4, space="PSUM") as pop:

            qTs = {}   # strip qc -> [3 tiles [P, QW] bf16]
            aoT = {}   # strip qc -> [3 tiles [P, QW] bf16]
            exs = {}   # (kc) -> ex pair tile for current strip
            pvb = {}   # live projB state per s

            def rope(pp, cs, sn):
                """psum natural QKV chunk [P, F] -> rotated bf16 sbuf tile."""
                p3 = pp.rearrange("p (h d) -> p h d", h=NH)
                c3 = cs.rearrange("p (h d) -> p h d", h=NH)
                s3 = sn.rearrange("p (h d) -> p h d", h=NH)
                t1 = rp_pool.tile([P, F], F32, tag="t1", name="t1")
                nc.vector.tensor_mul(t1[:], pp[:], cs[:])
                t2 = rp_pool.tile([P, F], F32, tag="t2", name="t2")
                t23 = t2.rearrange("p (h d) -> p h d", h=NH)
                nc.vector.tensor_mul(t23[:, :, 0:32], p3[:, :, 32:64],
                                     s3[:, :, 0:32])
                nc.vector.tensor_mul(t23[:, :, 32:64], p3[:, :, 0:32],
                                     s3[:, :, 32:64])
                qr = qr_pool.tile([P, F], F16, tag="qr", name="qr")
                nc.vector.tensor_add(qr[:], t1[:], t2[:])
                return qr

            def emit_projA(s):
                sl = slice(s * P, (s + 1) * P)
                qc = s // NQ
                if s % 4 == 0:
                    qTs[qc] = [qTs_pool.tile([P, QW], F16, tag=f"qTs{m}",
                                             name=f"qTs{m}") for m in range(MC)]
                cs = ld_pool.tile([P, F], F16, tag="cos", name="cs")
                nc.sync.dma_start(cs[:], cos_d[sl, :])
                sn = ld_pool.tile([P, F], F16, tag="sin", name="sn")
                nc.sync.dma_start(sn[:], sin_d[sl, :])
                pq = mx.tile([P, F], F32, tag="mx", name="pq")
                for f in range(FC):
                    nc.tensor.matmul(pq[:], xT[f][:, sl], wqT[f][:],
                                     start=(f == 0), stop=(f == FC - 1))
                pk = mx.tile([P, F], F32, tag="mx", name="pk")
                for f in range(FC):
                    nc.tensor.matmul(pk[:], xT[f][:, sl], wkT[f][:],
                                     start=(f == 0), stop=(f == FC - 1))
                qr = rope(pq, cs, sn)
                return (s, sl, cs, sn, pk, qr)

            def emit_projB(state):
                s, sl, cs, sn, pk, qr = state
                qc, scol = s // NQ, (s % 4) * P
                pv_ = mx.tile([P, F], F32, tag="mx", name="pv")
                for f in range(FC):
                    nc.tensor.matmul(pv_[:], xT[f][:, sl], wvT[f][:],
                                     start=(f == 0), stop=(f == FC - 1))
                kr = rope(pk, cs, sn)
                for m in range(MC):
                    tp = mx.tile([P, P], F16, tag="mx", name="tpq")
                    nc.tensor.transpose(tp[:], qr[:, m * P:(m + 1) * P],
                                        eye_sb[:])
                    nc.gpsimd.tensor_copy(qTs[qc][m][:, scol:scol + P], tp[:])
                for m in range(MC):
                    tp = mx.tile([P, P], F16, tag="mx", name="tpk")
                    nc.tensor.transpose(tp[:], kr[:, m * P:(m + 1) * P],
                                        eye_sb[:])
                    nc.gpsimd.tensor_copy(kT[m][:, sl], tp[:])
                v3 = vo[s].rearrange("p (h e) -> p h e", h=NH)
                p3 = pv_.rearrange("p (h d) -> p h d", h=NH)
                nc.gpsimd.tensor_copy(v3[:, :, 0:64], p3[:])

            def emit_scores(qc, kc, m):
                """head-pair m scores for block kc of strip qc + exp."""
                q0, k0 = qc * QW, kc * P
                off = max(0, k0 - q0)
                pair = []
                for par in range(2):
                    d0 = 64 * par
                    lhsT = kT[m][d0:d0 + 64, k0:k0 + P]
                    rhs = qTs[qc][m]
                    sp = scp.tile([P, QW], F32, tag="sc", name="sp")
                    if k0 >= q0:  # diagonal block: fold mask into PE
                        nc.tensor.matmul(sp[:, off:off + P],
                                         lhsT, rhs[d0:d0 + 64, off:off + P],
                                         start=True, stop=False)
                        nc.tensor.matmul(sp[:, off:off + P],
                                         eye_sb[:], tri_sb[:],
                                         start=False, stop=True)
                        if off + P < QW:
                            nc.tensor.matmul(sp[:, off + P:QW],
                                             lhsT, rhs[d0:d0 + 64, off + P:QW],
                                             start=True, stop=True)
                    else:
                        nc.tensor.matmul(sp[:],
                                         lhsT, rhs[d0:d0 + 64, :],
                                         start=True, stop=True)
                    ex = ex_pool.tile([P, QW], F16, tag="ex", name="ex")
                    nc.scalar.activation(ex[:, off:QW], sp[:, off:QW],
                                         AF.Exp, scale=0.125)
                    pair.append(ex)
                exs[(kc, m)] = pair

            def emit_pv(qc, kc, m, po4):
                """consume ex[(kc, m)]: accumulate into every live q-tile."""
                pair = exs.pop((kc, m))
                for qt in range(max(0, kc - NQ * qc), NQ):
                    t = NQ * qc + qt
                    for par in range(2):
                        h = 2 * m + par
                        lhsT = pair[par][:, qt * P:(qt + 1) * P]
                        nc.tensor.matmul(po4[qt][:, h * 65:h * 65 + 65],
                                         lhsT, vo[kc][:, h * 65:h * 65 + 65],
                                         start=(kc == 0), stop=(kc == t))

            def emit_norm(qc, qt, po):
                if qt == 0:
                    aoT[qc] = [ao_pool.tile([P, QW], F16, tag=f"aoT{m}",
                                            name=f"aoT{m}") for m in range(MC)]
                po3 = po.rearrange("p (h e) -> p h e", h=NH)
                inv = iv_pool.tile([P, NH], F32, tag="inv", name="inv")
                with nc.allow_low_precision(reason="softmax sums"):
                    nc.vector.reciprocal(inv[:], po3[:, :, 64])
                an = an_pool.tile([P, F], F16, tag="an", name="an")
                for h in range(NH):
                    nc.vector.tensor_scalar_mul(an[:, h * D:(h + 1) * D],
                                                po3[:, h, 0:D],
                                                inv[:, h:h + 1])
                for m in range(MC):
                    tp = mx.tile([P, P], F16, tag="mx", name="tpa")
                    nc.tensor.transpose(tp[:], an[:, m * P:(m + 1) * P],
                                        eye_sb[:])
                    nc.vector.tensor_copy(aoT[qc][m][:, qt * P:(qt + 1) * P],
                                          tp[:])

            def emit_oproj(qc, qt):
                s0 = (NQ * qc + qt) * P
                for half in range(2):
                    c0 = half * F
                    fin = mx.tile([P, F], F32, tag="mx", name="fin")
                    for m in range(MC):
                        nc.tensor.matmul(fin[:],
                                         aoT[qc][m][:, qt * P:(qt + 1) * P],
                                         woT[m][:, c0:c0 + F],
                                         start=(m == 0), stop=(m == MC - 1))
                    ob = ob_pool.tile([P, F], F32, tag="ob", name="ob")
                    nc.gpsimd.tensor_copy(ob[:], fin[:])
                    nc.sync.dma_start(out_d[s0:s0 + P, c0:c0 + F], ob[:])

            # ---- emission schedule ----
            # fills: PE work units consumed 1-2 per scores block to keep the
            # PE busy during the Act(exp)-bound late strips. Proj A/B pairs
            # go in dependency order; o_proj units are appended as strips
            # complete and drain after the proj units run out.
            fills = deque()
            for s in range(4, SC):
                fills.append(("A", s))
                fills.append(("B", s))

            def pop_fill():
                if not fills:
                    return
                kind, a = fills.popleft()
                if kind == "A":
                    pvb[a] = emit_projA(a)
                elif kind == "B":
                    emit_projB(pvb.pop(a))
                else:
                    emit_oproj(*a)

            for s in range(4):
                st = emit_projA(s)
                emit_projB(st)
            for c in range(1, NQ):
                cl = slice(c * QW, (c + 1) * QW)
                for f in range(FC):
                    nc.sync.dma_start(xT[f][:, cl], xT_d[f * P:(f + 1) * P, cl])

            for qc in range(NQ):
                exs.clear()
                last = NQ * qc + 3
                po4 = [pop.tile([P, NH * 65], F32, tag="po", name="po")
                       for _ in range(NQ)]
                for kc in range(last + 1):
                    pop_fill()
                    if qc > 0:
                        pop_fill()
                    for m in range(MC):
                        emit_scores(qc, kc, m)
                        if kc > 0:
                            emit_pv(qc, kc - 1, m, po4)
                    qt_done = kc - 1 - NQ * qc
                    if 0 <= qt_done < NQ:
                        emit_norm(qc, qt_done, po4[qt_done])
                        fills.append(("O", (qc, qt_done)))
                for m in range(MC):
                    emit_pv(qc, last, m, po4)
                emit_norm(qc, NQ - 1, po4[NQ - 1])
                fills.append(("O", (qc, NQ - 1)))
            while fills:
                pop_fill()
    nc.compile()
    return nc


def _rope_tables():
    inv_freq = 1.0 / (ROPE_THETA ** (np.arange(0, D, 2, dtype=np.float32) / D))
    t = np.arange(S, dtype=np.float32)
    freqs = np.outer(t, inv_freq)                       # [S, 32]
    emb = np.concatenate([freqs, freqs], axis=-1)       # [S, 64]
    cos = np.cos(emb).astype(np.float32)
    sin = np.sin(emb).astype(np.float32)
    sin_signed = sin.copy()
    sin_signed[:, 0:32] *= -1.0                         # fold rotate_half sign
    cos6 = np.tile(cos, (1, NH)).astype(np.float16)
    sin6 = np.tile(sin_signed, (1, NH)).astype(np.float16)
    return np.ascontiguousarray(cos6), np.ascontiguousarray(sin6)


_STATE = {}


def _get_program():
    if "nc" not in _STATE:
        _STATE["nc"] = build_program()
    return _STATE["nc"]


def _make_in_maps(hidden_states, Wq, Wk, Wv, Wo):
    hs = np.asarray(hidden_states, dtype=np.float32)
    Wq = np.asarray(Wq, dtype=np.float32)
    Wk = np.asarray(Wk, dtype=np.float32)
    Wv = np.asarray(Wv, dtype=np.float32)
    Wo = np.asarray(Wo, dtype=np.float32)

    cos6, sin6 = _rope_tables()
    trineg = (NEG * np.tril(np.ones((P, P), dtype=np.float32), -1)
              ).astype(np.float16)
    eye = np.eye(P, dtype=np.float16)

    in_maps = []
    for c in range(N_CORES):
        b, g = c // 2, c % 2
        cols = slice(g * F, (g + 1) * F)
        in_maps.append({
            "xT": np.ascontiguousarray(hs[b].T).astype(np.float16),
            "wqT": np.ascontiguousarray(Wq[cols, :].T).astype(np.float16),
            "wkT": np.ascontiguousarray(Wk[cols, :].T).astype(np.float16),
            "wvT": np.ascontiguousarray(Wv[cols, :].T).astype(np.float16),
            "woT": np.ascontiguousarray(Wo[:, cols].T).astype(np.float16),
            "cos6": cos6,
            "sin6": sin6,
            "trineg": trineg,
            "eye": eye,
        })
    return in_maps


def run(hidden_states, Wq, Wk, Wv, Wo, trace=False, **trace_kw):
    nc = _get_program()
    in_maps = _make_in_maps(hidden_states, Wq, Wk, Wv, Wo)
    res = run_bass_kernel_spmd(nc, in_maps, core_ids=list(range(N_CORES)),
                               trace=trace, **trace_kw)
    B = 4
    out = np.empty((B, S, HID), dtype=np.float32)
    for b in range(B):
        out[b] = res.results[2 * b]["out"] + res.results[2 * b + 1]["out"]
    return out, res


def kernel(hidden_states, Wq, Wk, Wv, Wo):
    out, _ = run(hidden_states, Wq, Wk, Wv, Wo,
                 trace=bool(int(os.environ.get("KERNEL_TRACE", "0"))))
    return out


# revision 11
# speedup vs baseline: 1.2802x; 1.0258x over previous
"""Trainium2 Bass kernel for HNet attention (B=4, S=2048, H=768, 12 heads, RoPE, causal).

Sharding: 8 cores = 4 batches x 2 head-groups (6 heads each).
Wq/Wk/Wv split column-wise (head axis), Wo row-wise; host sums the two
partial o_proj outputs per batch (the "all-reduce" done at gather time).

Per-core dataflow (v4 — fp16 inputs, natural-layout PV, packed DMAs):
  xT [768,2048] fp16 (host-packed by column-chunk) --PE--> Q,K,V natural
  RoPE on Q,K in natural layout (DVE t1 / Pool t2), PE-transpose fp16
  scoresT[k,q] = kT.T @ qT per (head-pair, par) with PE row groups; causal
    mask folded into the PE as an accumulate-matmul of a -30000 triangle
  exp on ScalarE -> ex fp16; PV natural: po[q, 65] += ex.T @ [V_h | 1]
  (col 64 = softmax sums); normalize via reciprocal + per-head tensor_scalar;
  deferred fill: PE-transpose -> aoT, o_proj fin = aoT.T @ woT, strip store.
"""

import os
import sys

import numpy as np

sys.path.insert(0, "/opt/trn_rl_repo")

from collections import deque
from contextlib import ExitStack

import concourse.bacc as bacc
import concourse.tile as tile
from concourse import mybir
from concourse.bass_utils import run_bass_kernel_spmd

S = 2048
HID = 768
NH = 6            # heads per core
D = 64
F = NH * D        # 384 per-core feature slice
P = 128
SC = S // P       # 16 s-tiles
FC = HID // P     # 6 contraction chunks
MC = F // P       # 3 head-pair chunks
QW = 512          # q strip width
NQ = S // QW      # 4 strips
N_CORES = 8
ROPE_THETA = 10000.0
NEG = -30000.0

F32 = mybir.dt.float32
F16 = mybir.dt.float16
AF = mybir.ActivationFunctionType

XW = NQ * FC * QW      # packed xT width 12288
WW = FC * 3 * F        # packed wqkv width 6912
CW = SC * 2 * F        # packed cos|sin width 12288


def build_program():
    nc = bacc.Bacc("TRN2", target_bir_lowering=False, debug=False,
                   num_devices=N_CORES)

    xTp_d = nc.dram_tensor("xTp", [P, XW], F16, kind="ExternalInput").ap()
    wp_d = nc.dram_tensor("wp", [P, WW], F16, kind="ExternalInput").ap()
    woT_d = nc.dram_tensor("woT", [P, MC * HID], F16, kind="ExternalInput").ap()
    csn_d = nc.dram_tensor("csn", [P, CW], F16, kind="ExternalInput").ap()
    tri_d = nc.dram_tensor("trineg", [P, P], F16, kind="ExternalInput").ap()
    eye_d = nc.dram_tensor("eye", [P, P], F16, kind="ExternalInput").ap()
    out_d = nc.dram_tensor("out", [S, HID], F32, kind="ExternalOutput").ap()

    with tile.TileContext(nc) as tc, ExitStack() as ctx:
        const_pool = ctx.enter_context(tc.tile_pool(name="const", bufs=1))
        eye_sb = const_pool.tile([P, P], F16, tag="eye")
        nc.sync.dma_start(eye_sb[:], eye_d[:])
        tri_sb = const_pool.tile([P, P], F16, tag="tri")
        nc.sync.dma_start(tri_sb[:], tri_d[:])

        # ---- persistent SBUF; DMA order feeds the prologue first ----
        xw_pool = ctx.enter_context(tc.tile_pool(name="xw", bufs=1))
        wp = xw_pool.tile([P, WW], F16, tag="wp")
        xTp = xw_pool.tile([P, XW], F16, tag="xTp")
        csn = xw_pool.tile([P, CW], F16, tag="csn")
        woT = xw_pool.tile([P, MC * HID], F16, tag="woT")
        nc.sync.dma_start(wp[:, 0:WW // 2], wp_d[:, 0:WW // 2])
        nc.sync.dma_start(wp[:, WW // 2:WW], wp_d[:, WW // 2:WW])
        nc.sync.dma_start(xTp[:, 0:XW // 4], xTp_d[:, 0:XW // 4])
        nc.sync.dma_start(csn[:, 0:CW // 4], csn_d[:, 0:CW // 4])
        nc.sync.dma_start(woT[:], woT_d[:])

        def wslice(kind, f):  # 0=q 1=k 2=v
            c0 = f * 3 * F + kind * F
            return wp[:, c0:c0 + F]

        def xslice(f, s):
            c0 = (s // 4) * (FC * QW) + f * QW + (s % 4) * P
            return xTp[:, c0:c0 + P]

        kT_pool = ctx.enter_context(tc.tile_pool(name="kTp", bufs=1))
        kT = [kT_pool.tile([P, S], F16, tag=f"kT{m}", name=f"kT{m}")
              for m in range(MC)]
        v_pool = ctx.enter_context(tc.tile_pool(name="vp", bufs=1))
        vo = [v_pool.tile([P, NH * 65], F16, tag=f"v{s}", name=f"v{s}")
              for s in range(SC)]
        for s in range(SC):
            v3 = vo[s].rearrange("p (h e) -> p h e", h=NH)
            nc.gpsimd.memset(v3[:, :, 64], 1.0)

        with tc.tile_pool(name="rp", bufs=2) as rp_pool, \
             tc.tile_pool(name="qr", bufs=4) as qr_pool, \
             tc.tile_pool(name="qTs", bufs=4) as qTs_pool, \
             tc.tile_pool(name="ao", bufs=3) as ao_pool, \
             tc.tile_pool(name="ex", bufs=12) as ex_pool, \
             tc.tile_pool(name="an", bufs=4) as an_pool, \
             tc.tile_pool(name="iv", bufs=4) as iv_pool, \
             tc.tile_pool(name="ob", bufs=2) as ob_pool, \
             tc.tile_pool(name="mx", bufs=2, space="PSUM") as mx, \
             tc.tile_pool(name="sc", bufs=2, space="PSUM") as scp, \
             tc.tile_pool(name="po", bufs=4, space="PSUM") as pop:

            qTs = {}   # strip qc -> [3 tiles [P, QW] f16]
            aoT = {}   # strip qc -> [3 tiles [P, QW] f16]
            ans = {}   # (qc, qt) -> normalized ao_nat tile
            exs = {}   # (kc, m) -> (ex0, ex1)
            obs = {}   # strip qc -> packed output staging tile
            pvb = {}   # live projB state per s

            def rope(pp, s):
                """psum natural QKV chunk [P, F] -> rotated fp16 sbuf tile.
                t1 = x*cos on DVE; t2 = rot_half(x)*sin_signed on Pool."""
                cs = csn[:, s * 2 * F:s * 2 * F + F]
                sn = csn[:, s * 2 * F + F:s * 2 * F + 2 * F]
                p3 = pp.rearrange("p (h d) -> p h d", h=NH)
                s3 = sn.rearrange("p (h d) -> p h d", h=NH)
                t1 = rp_pool.tile([P, F], F32, tag="t1", name="t1")
                nc.vector.tensor_mul(t1[:], pp[:], cs[:])
                t2 = rp_pool.tile([P, F], F32, tag="t2", name="t2")
                t23 = t2.rearrange("p (h d) -> p h d", h=NH)
                nc.gpsimd.tensor_mul(t23[:, :, 0:32], p3[:, :, 32:64],
                                     s3[:, :, 0:32])
                nc.gpsimd.tensor_mul(t23[:, :, 32:64], p3[:, :, 0:32],
                                     s3[:, :, 32:64])
                qr = qr_pool.tile([P, F], F16, tag="qr", name="qr")
                nc.vector.tensor_add(qr[:], t1[:], t2[:])
                return qr

            def emit_projA(s):
                qc = s // NQ
                if s % 4 == 0:
                    qTs[qc] = [qTs_pool.tile([P, QW], F16, tag=f"qTs{m}",
                                             name=f"qTs{m}") for m in range(MC)]
                pq = mx.tile([P, F], F32, tag="mx", name="pq")
                for f in range(FC):
                    nc.tensor.matmul(pq[:], xslice(f, s), wslice(0, f),
                                     start=(f == 0), stop=(f == FC - 1))
                pk = mx.tile([P, F], F32, tag="mx", name="pk")
                for f in range(FC):
                    nc.tensor.matmul(pk[:], xslice(f, s), wslice(1, f),
                                     start=(f == 0), stop=(f == FC - 1))
                qr = rope(pq, s)
                return (s, pk, qr)

            def emit_projB(state):
                s, pk, qr = state
                qc, scol = s // NQ, (s % 4) * P
                pv_ = mx.tile([P, F], F32, tag="mx", name="pv")
                for f in range(FC):
                    nc.tensor.matmul(pv_[:], xslice(f, s), wslice(2, f),
                                     start=(f == 0), stop=(f == FC - 1))
                kr = rope(pk, s)
                for m in range(MC):
                    tp = mx.tile([P, P], F16, tag="mx", name="tpq")
                    nc.tensor.transpose(tp[:], qr[:, m * P:(m + 1) * P],
                                        eye_sb[:])
                    nc.gpsimd.tensor_copy(qTs[qc][m][:, scol:scol + P], tp[:])
                for m in range(MC):
                    tp = mx.tile([P, P], F16, tag="mx", name="tpk")
                    nc.tensor.transpose(tp[:], kr[:, m * P:(m + 1) * P],
                                        eye_sb[:])
                    nc.gpsimd.tensor_copy(kT[m][:, s * P:(s + 1) * P], tp[:])
                v3 = vo[s].rearrange("p (h e) -> p h e", h=NH)
                p3 = pv_.rearrange("p (h d) -> p h d", h=NH)
                nc.gpsimd.tensor_copy(v3[:, :, 0:64], p3[:])

            def emit_scores(qc, kc, m):
                """head-pair m scores for block kc of strip qc + exp."""
                q0, k0 = qc * QW, kc * P
                off = max(0, k0 - q0)
                pair = []
                for par in range(2):
                    d0 = 64 * par
                    lhsT = kT[m][d0:d0 + 64, k0:k0 + P]
                    rhs = qTs[qc][m]
                    sp = scp.tile([P, QW], F32, tag="sc", name="sp")
                    if k0 >= q0:  # diagonal block: fold mask into PE
                        nc.tensor.matmul(sp[:, off:off + P],
                                         lhsT, rhs[d0:d0 + 64, off:off + P],
                                         start=True, stop=False)
                        nc.tensor.matmul(sp[:, off:off + P],
                                         eye_sb[:], tri_sb[:],
                                         start=False, stop=True)
                        if off + P < QW:
                            nc.tensor.matmul(sp[:, off + P:QW],
                                             lhsT, rhs[d0:d0 + 64, off + P:QW],
                                             start=True, stop=True)
                    else:
                        nc.tensor.matmul(sp[:],
                                         lhsT, rhs[d0:d0 + 64, :],
                                         start=True, stop=True)
                    ex = ex_pool.tile([P, QW], F16, tag="ex", name="ex")
                    nc.scalar.activation(ex[:, off:QW], sp[:, off:QW],
                                         AF.Exp, scale=0.125)
                    pair.append(ex)
                exs[(kc, m)] = pair

            def emit_pv(qc, kc, m, po4):
                """consume ex[(kc, m)]: accumulate into every live q-tile."""
                pair = exs.pop((kc, m))
                for qt in range(max(0, kc - NQ * qc), NQ):
                    t = NQ * qc + qt
                    for par in range(2):
                        h = 2 * m + par
                        lhsT = pair[par][:, qt * P:(qt + 1) * P]
                        nc.tensor.matmul(po4[qt][:, h * 65:h * 65 + 65],
                                         lhsT, vo[kc][:, h * 65:h * 65 + 65],
                                         start=(kc == 0), stop=(kc == t))

            def emit_norm(qc, qt, po):
                """reciprocal + per-head scale; PE transposes deferred."""
                po3 = po.rearrange("p (h e) -> p h e", h=NH)
                inv = iv_pool.tile([P, NH], F32, tag="inv", name="inv")
                with nc.allow_low_precision(reason="softmax sums"):
                    nc.vector.reciprocal(inv[:], po3[:, :, 64])
                an = an_pool.tile([P, F], F16, tag="an", name="an")
                for h in range(NH):
                    nc.vector.tensor_scalar_mul(an[:, h * D:(h + 1) * D],
                                                po3[:, h, 0:D],
                                                inv[:, h:h + 1])
                ans[(qc, qt)] = an

            def emit_oproj(qc, qt):
                """deferred PE fill: transpose ao_nat -> aoT, fin, store."""
                if qt == 0:
                    aoT[qc] = [ao_pool.tile([P, QW], F16, tag=f"aoT{m}",
                                            name=f"aoT{m}") for m in range(MC)]
                    obs[qc] = ob_pool.tile([P, NQ * HID], F32, tag="ob",
                                           name="ob")
                an = ans.pop((qc, qt))
                for m in range(MC):
                    tp = mx.tile([P, P], F16, tag="mx", name="tpa")
                    nc.tensor.transpose(tp[:], an[:, m * P:(m + 1) * P],
                                        eye_sb[:])
                    nc.vector.tensor_copy(aoT[qc][m][:, qt * P:(qt + 1) * P],
                                          tp[:])
                ob = obs[qc]
                for half in range(2):
                    c0 = half * F
                    fin = mx.tile([P, F], F32, tag="mx", name="fin")
                    for m in range(MC):
                        nc.tensor.matmul(fin[:],
                                         aoT[qc][m][:, qt * P:(qt + 1) * P],
                                         woT[:, m * HID + c0:m * HID + c0 + F],
                                         start=(m == 0), stop=(m == MC - 1))
                    nc.gpsimd.tensor_copy(ob[:, qt * HID + c0:
                                             qt * HID + c0 + F], fin[:])
                if qt == NQ - 1:
                    dst = out_d[qc * QW:(qc + 1) * QW, :].rearrange(
                        "(t p) c -> p t c", p=P)
                    src = ob.rearrange("p (t c) -> p t c", t=NQ)
                    nc.sync.dma_start(dst, src)

            # ---- emission schedule ----
            fills = deque()
            for s in range(4, SC):
                fills.append(("A", s))
                fills.append(("B", s))

            def pop_fill():
                if not fills:
                    return
                kind, a = fills.popleft()
                if kind == "A":
                    pvb[a] = emit_projA(a)
                elif kind == "B":
                    emit_projB(pvb.pop(a))
                else:
                    emit_oproj(*a)

            for s in range(4):
                st = emit_projA(s)
                emit_projB(st)
            for c in range(1, NQ):
                nc.sync.dma_start(xTp[:, c * (XW // 4):(c + 1) * (XW // 4)],
                                  xTp_d[:, c * (XW // 4):(c + 1) * (XW // 4)])
                nc.sync.dma_start(csn[:, c * (CW // 4):(c + 1) * (CW // 4)],
                                  csn_d[:, c * (CW // 4):(c + 1) * (CW // 4)])

            for qc in range(NQ):
                exs.clear()
                last = NQ * qc + 3
                po4 = [pop.tile([P, NH * 65], F32, tag="po", name="po")
                       for _ in range(NQ)]
                for kc in range(last + 1):
                    pop_fill()
                    if qc > 0:
                        pop_fill()
                    for m in range(MC):
                        emit_scores(qc, kc, m)
                        if kc > 0:
                            emit_pv(qc, kc - 1, m, po4)
                    qt_done = kc - 1 - NQ * qc
                    if 0 <= qt_done < NQ:
                        emit_norm(qc, qt_done, po4[qt_done])
                        fills.append(("O", (qc, qt_done)))
                for m in range(MC):
                    emit_pv(qc, last, m, po4)
                emit_norm(qc, NQ - 1, po4[NQ - 1])
                fills.append(("O", (qc, NQ - 1)))
            while fills:
                pop_fill()
    nc.compile()
    return nc


def _rope_tables():
    inv_freq = 1.0 / (ROPE_THETA ** (np.arange(0, D, 2, dtype=np.float32) / D))
    t = np.arange(S, dtype=np.float32)
    freqs = np.outer(t, inv_freq)                       # [S, 32]
    emb = np.concatenate([freqs, freqs], axis=-1)       # [S, 64]
    cos = np.cos(emb).astype(np.float32)
    sin = np.sin(emb).astype(np.float32)
    sin_signed = sin.copy()
    sin_signed[:, 0:32] *= -1.0                         # fold rotate_half sign
    cos6 = np.tile(cos, (1, NH))                        # [S, 384]
    sin6 = np.tile(sin_signed, (1, NH))
    # pack [cos | sin] per s-tile: [128, 16*768]
    both = np.concatenate(
        [cos6.reshape(SC, P, F), sin6.reshape(SC, P, F)], axis=2)  # [16,128,768]
    return np.ascontiguousarray(
        both.transpose(1, 0, 2).reshape(P, CW)).astype(np.float16)


_STATE = {}


def _get_program():
    if "nc" not in _STATE:
        _STATE["nc"] = build_program()
    return _STATE["nc"]


def _pack_x(xT):
    """[768, 2048] -> [128, 12288] with cols (c, f, 512)."""
    v = xT.reshape(FC, P, NQ, QW)               # f, p, c, col
    return np.ascontiguousarray(
        v.transpose(1, 2, 0, 3).reshape(P, XW))  # p, (c f col)


def _pack_w(Wq, Wk, Wv, cols):
    """3x [768, 384] (transposed slices) -> [128, 6912] cols (f, kind, 384)."""
    ws = [np.asarray(W[cols, :].T, dtype=np.float32).reshape(FC, P, F)
          for W in (Wq, Wk, Wv)]
    stk = np.stack(ws, axis=2)                   # f, p, kind, 384
    return np.ascontiguousarray(stk.transpose(1, 0, 2, 3).reshape(P, WW))


def _make_in_maps(hidden_states, Wq, Wk, Wv, Wo):
    hs = np.asarray(hidden_states, dtype=np.float32)
    Wq = np.asarray(Wq, dtype=np.float32)
    Wk = np.asarray(Wk, dtype=np.float32)
    Wv = np.asarray(Wv, dtype=np.float32)
    Wo = np.asarray(Wo, dtype=np.float32)

    csn = _rope_tables()
    trineg = (NEG * np.tril(np.ones((P, P), dtype=np.float32), -1)
              ).astype(np.float16)
    eye = np.eye(P, dtype=np.float16)

    in_maps = []
    for c in range(N_CORES):
        b, g = c // 2, c % 2
        cols = slice(g * F, (g + 1) * F)
        woT = np.asarray(Wo[:, cols].T, dtype=np.float32)    # [384, 768]
        woTp = np.ascontiguousarray(
            woT.reshape(MC, P, HID).transpose(1, 0, 2).reshape(P, MC * HID))
        in_maps.append({
            "xTp": _pack_x(hs[b].T).astype(np.float16),
            "wp": _pack_w(Wq, Wk, Wv, cols).astype(np.float16),
            "woT": woTp.astype(np.float16),
            "csn": csn,
            "trineg": trineg,
            "eye": eye,
        })
    return in_maps


def run(hidden_states, Wq, Wk, Wv, Wo, trace=False, **trace_kw):
    nc = _get_program()
    in_maps = _make_in_maps(hidden_states, Wq, Wk, Wv, Wo)
    res = run_bass_kernel_spmd(nc, in_maps, core_ids=list(range(N_CORES)),
                               trace=trace, **trace_kw)
    B = 4
    out = np.empty((B, S, HID), dtype=np.float32)
    for b in range(B):
        out[b] = res.results[2 * b]["out"] + res.results[2 * b + 1]["out"]
    return out, res


def kernel(hidden_states, Wq, Wk, Wv, Wo):
    out, _ = run(hidden_states, Wq, Wk, Wv, Wo,
                 trace=bool(int(os.environ.get("KERNEL_TRACE", "0"))))
    return out


# revision 12
# speedup vs baseline: 1.3961x; 1.0905x over previous
"""Trainium2 Bass kernel for HNet attention (B=4, S=2048, H=768, 12 heads, RoPE, causal).

Sharding: 8 cores = 4 batches x 2 head-groups (6 heads each).
Wq/Wk/Wv split column-wise (head axis), Wo row-wise; host sums the two
partial o_proj outputs per batch (the "all-reduce" done at gather time).

Per-core dataflow (v4 — fp16 inputs, natural-layout PV, packed DMAs):
  xT [768,2048] fp16 (host-packed by column-chunk) --PE--> Q,K,V natural
  RoPE on Q,K in natural layout (DVE t1 / Pool t2), PE-transpose fp16
  scoresT[k,q] = kT.T @ qT per (head-pair, par) with PE row groups; causal
    mask folded into the PE as an accumulate-matmul of a -30000 triangle
  exp on ScalarE -> ex fp16; PV natural: po[q, 65] += ex.T @ [V_h | 1]
  (col 64 = softmax sums); normalize via reciprocal + per-head tensor_scalar;
  deferred fill: PE-transpose -> aoT, o_proj fin = aoT.T @ woT, strip store.
"""

import os
import sys

import numpy as np

sys.path.insert(0, "/opt/trn_rl_repo")

from collections import deque
from contextlib import ExitStack

import concourse.bacc as bacc
import concourse.tile as tile
from concourse import mybir
from concourse.bass_utils import run_bass_kernel_spmd

S = 2048
HID = 768
NH = 6            # heads per core
D = 64
F = NH * D        # 384 per-core feature slice
P = 128
SC = S // P       # 16 s-tiles
FC = HID // P     # 6 contraction chunks
MC = F // P       # 3 head-pair chunks
QW = 512          # q strip width
NQ = S // QW      # 4 strips
N_CORES = 8
ROPE_THETA = 10000.0
NEG = -30000.0

F32 = mybir.dt.float32
F16 = mybir.dt.float16
F8 = mybir.dt.float8e4
AF = mybir.ActivationFunctionType

XW = NQ * FC * QW      # packed xT width 12288
WW = FC * 3 * F        # packed wqkv width 6912
CW = SC * 2 * F        # packed cos|sin width 12288


def build_program():
    nc = bacc.Bacc("TRN2", target_bir_lowering=False, debug=False,
                   num_devices=N_CORES)

    xTp_d = nc.dram_tensor("xTp", [P, XW], F16, kind="ExternalInput").ap()
    wp_d = nc.dram_tensor("wp", [P, WW], F16, kind="ExternalInput").ap()
    woT_d = nc.dram_tensor("woT", [P, MC * HID], F16, kind="ExternalInput").ap()
    csn_d = nc.dram_tensor("csn", [P, CW], F16, kind="ExternalInput").ap()
    tri_d = nc.dram_tensor("trineg", [P, P], F16, kind="ExternalInput").ap()
    eye_d = nc.dram_tensor("eye", [P, P], F16, kind="ExternalInput").ap()
    out_d = nc.dram_tensor("out", [S, HID], F32, kind="ExternalOutput").ap()

    with tile.TileContext(nc) as tc, ExitStack() as ctx:
        const_pool = ctx.enter_context(tc.tile_pool(name="const", bufs=1))
        eye_sb = const_pool.tile([P, P], F16, tag="eye")
        nc.sync.dma_start(eye_sb[:], eye_d[:])
        tri_sb = const_pool.tile([P, P], F16, tag="tri")
        nc.sync.dma_start(tri_sb[:], tri_d[:])

        # ---- persistent SBUF; DMA order feeds the prologue first ----
        xw_pool = ctx.enter_context(tc.tile_pool(name="xw", bufs=1))
        wp = xw_pool.tile([P, WW], F16, tag="wp")
        xTp = xw_pool.tile([P, XW], F16, tag="xTp")
        csn = xw_pool.tile([P, CW], F16, tag="csn")
        woT = xw_pool.tile([P, MC * HID], F16, tag="woT")
        nc.sync.dma_start(wp[:, 0:WW // 2], wp_d[:, 0:WW // 2])
        nc.sync.dma_start(wp[:, WW // 2:WW], wp_d[:, WW // 2:WW])
        nc.sync.dma_start(xTp[:, 0:XW // 4], xTp_d[:, 0:XW // 4])
        nc.sync.dma_start(csn[:, 0:CW // 4], csn_d[:, 0:CW // 4])
        nc.sync.dma_start(woT[:], woT_d[:])

        def wslice(kind, f):  # 0=q 1=k 2=v
            c0 = f * 3 * F + kind * F
            return wp[:, c0:c0 + F]

        def xslice(f, s):
            c0 = (s // 4) * (FC * QW) + f * QW + (s % 4) * P
            return xTp[:, c0:c0 + P]

        kT_pool = ctx.enter_context(tc.tile_pool(name="kTp", bufs=1))
        kT = [kT_pool.tile([P, S], F16, tag=f"kT{m}", name=f"kT{m}")
              for m in range(MC)]
        v_pool = ctx.enter_context(tc.tile_pool(name="vp", bufs=1))
        vo = [v_pool.tile([P, NH * 65], F16, tag=f"v{s}", name=f"v{s}")
              for s in range(SC)]
        for s in range(SC):
            v3 = vo[s].rearrange("p (h e) -> p h e", h=NH)
            nc.gpsimd.memset(v3[:, :, 64], 1.0)

        with tc.tile_pool(name="rp", bufs=2) as rp_pool, \
             tc.tile_pool(name="qr", bufs=4) as qr_pool, \
             tc.tile_pool(name="qTs", bufs=2) as qTs_pool, \
             tc.tile_pool(name="ao", bufs=3) as ao_pool, \
             tc.tile_pool(name="ex", bufs=54) as ex_pool, \
             tc.tile_pool(name="an", bufs=3) as an_pool, \
             tc.tile_pool(name="iv", bufs=4) as iv_pool, \
             tc.tile_pool(name="ob", bufs=2) as ob_pool, \
             tc.tile_pool(name="mx", bufs=2, space="PSUM") as mx, \
             tc.tile_pool(name="sc", bufs=2, space="PSUM") as scp, \
             tc.tile_pool(name="po", bufs=2, space="PSUM") as pop:

            qTs = {}   # strip qc -> [3 tiles [P, QW] f16]
            aoT = {}   # strip qc -> [3 tiles [P, QW] f16]
            ans = {}   # (qc, qt) -> normalized ao_nat tile
            exs = {}   # (kc, m) -> (ex0, ex1)
            obs = {}   # strip qc -> packed output staging tile
            pvb = {}   # live projB state per s

            def rope(pp, s):
                """psum natural QKV chunk [P, F] -> rotated fp16 sbuf tile.
                t1 = x*cos on DVE; t2 = rot_half(x)*sin_signed on Pool."""
                cs = csn[:, s * 2 * F:s * 2 * F + F]
                sn = csn[:, s * 2 * F + F:s * 2 * F + 2 * F]
                p3 = pp.rearrange("p (h d) -> p h d", h=NH)
                s3 = sn.rearrange("p (h d) -> p h d", h=NH)
                t1 = rp_pool.tile([P, F], F32, tag="t1", name="t1")
                nc.vector.tensor_mul(t1[:], pp[:], cs[:])
                t2 = rp_pool.tile([P, F], F32, tag="t2", name="t2")
                t23 = t2.rearrange("p (h d) -> p h d", h=NH)
                nc.gpsimd.tensor_mul(t23[:, :, 0:32], p3[:, :, 32:64],
                                     s3[:, :, 0:32])
                nc.gpsimd.tensor_mul(t23[:, :, 32:64], p3[:, :, 0:32],
                                     s3[:, :, 32:64])
                qr = qr_pool.tile([P, F], F16, tag="qr", name="qr")
                nc.vector.tensor_add(qr[:], t1[:], t2[:])
                return qr

            def emit_projA(s):
                qc = s // NQ
                if s % 4 == 0:
                    qTs[qc] = [qTs_pool.tile([P, QW], F16, tag=f"qTs{m}",
                                             name=f"qTs{m}") for m in range(MC)]
                pq = mx.tile([P, F], F32, tag="mx", name="pq")
                for f in range(FC):
                    nc.tensor.matmul(pq[:], xslice(f, s), wslice(0, f),
                                     start=(f == 0), stop=(f == FC - 1))
                pk = mx.tile([P, F], F32, tag="mx", name="pk")
                for f in range(FC):
                    nc.tensor.matmul(pk[:], xslice(f, s), wslice(1, f),
                                     start=(f == 0), stop=(f == FC - 1))
                qr = rope(pq, s)
                return (s, pk, qr)

            def emit_projB(state):
                s, pk, qr = state
                qc, scol = s // NQ, (s % 4) * P
                pv_ = mx.tile([P, F], F32, tag="mx", name="pv")
                for f in range(FC):
                    nc.tensor.matmul(pv_[:], xslice(f, s), wslice(2, f),
                                     start=(f == 0), stop=(f == FC - 1))
                kr = rope(pk, s)
                for m in range(MC):
                    tp = mx.tile([P, P], F16, tag="mx", name="tpq")
                    nc.tensor.transpose(tp[:], qr[:, m * P:(m + 1) * P],
                                        eye_sb[:])
                    nc.gpsimd.tensor_copy(qTs[qc][m][:, scol:scol + P], tp[:])
                for m in range(MC):
                    tp = mx.tile([P, P], F16, tag="mx", name="tpk")
                    nc.tensor.transpose(tp[:], kr[:, m * P:(m + 1) * P],
                                        eye_sb[:])
                    nc.gpsimd.tensor_copy(kT[m][:, s * P:(s + 1) * P], tp[:])
                v3 = vo[s].rearrange("p (h e) -> p h e", h=NH)
                p3 = pv_.rearrange("p (h d) -> p h d", h=NH)
                nc.gpsimd.tensor_copy(v3[:, :, 0:64], p3[:])

            def emit_scores(qc, kc, m):
                """head-pair m scores for block kc of strip qc + merged exp."""
                q0, k0 = qc * QW, kc * P
                off = max(0, k0 - q0)
                sp = scp.tile([P, 2 * QW], F32, tag="sc", name="sp")
                for par in range(2):
                    b = par * QW
                    d0 = 64 * par
                    lhsT = kT[m][d0:d0 + 64, k0:k0 + P]
                    rhs = qTs[qc][m]
                    if k0 >= q0:  # diagonal block: fold mask into PE
                        nc.tensor.matmul(sp[:, b + off:b + off + P],
                                         lhsT, rhs[d0:d0 + 64, off:off + P],
                                         start=True, stop=False)
                        nc.tensor.matmul(sp[:, b + off:b + off + P],
                                         eye_sb[:], tri_sb[:],
                                         start=False, stop=True)
                        if off + P < QW:
                            nc.tensor.matmul(sp[:, b + off + P:b + QW],
                                             lhsT, rhs[d0:d0 + 64, off + P:QW],
                                             start=True, stop=True)
                    else:
                        nc.tensor.matmul(sp[:, b:b + QW],
                                         lhsT, rhs[d0:d0 + 64, :],
                                         start=True, stop=True)
                ex = ex_pool.tile([P, 2 * QW], F8, tag="ex", name="ex")
                sp3 = sp.rearrange("p (r c) -> p r c", r=2)
                ex3 = ex.rearrange("p (r c) -> p r c", r=2)
                nc.scalar.activation(ex3[:, :, off:QW], sp3[:, :, off:QW],
                                     AF.Exp, scale=0.125)
                exs[(kc, m)] = ex

            def emit_pv(qc, qt):
                """qt-serial PV: accumulate po over all kc of the strip."""
                t = NQ * qc + qt
                po = pop.tile([P, NH * 65], F32, tag="po", name="po")
                for kc in range(t + 1):
                    for m in range(MC):
                        ex = exs[(kc, m)]
                        for par in range(2):
                            h = 2 * m + par
                            lhsT = ex[:, par * QW + qt * P:
                                      par * QW + (qt + 1) * P]
                            nc.tensor.matmul(po[:, h * 65:h * 65 + 65],
                                             lhsT, vo[kc][:, h * 65:h * 65 + 65],
                                             start=(kc == 0), stop=(kc == t))
                return po

            def emit_norm(qc, qt, po):
                """reciprocal + per-head scale; PE transposes deferred."""
                po3 = po.rearrange("p (h e) -> p h e", h=NH)
                inv = iv_pool.tile([P, NH], F32, tag="inv", name="inv")
                with nc.allow_low_precision(reason="softmax sums"):
                    nc.vector.reciprocal(inv[:], po3[:, :, 64])
                an = an_pool.tile([P, F], F16, tag="an", name="an")
                for h in range(NH):
                    nc.vector.tensor_scalar_mul(an[:, h * D:(h + 1) * D],
                                                po3[:, h, 0:D],
                                                inv[:, h:h + 1])
                ans[(qc, qt)] = an

            def emit_oproj(qc, qt):
                """deferred PE fill: transpose ao_nat -> aoT, fin, store."""
                if qt == 0:
                    aoT[qc] = [ao_pool.tile([P, QW], F16, tag=f"aoT{m}",
                                            name=f"aoT{m}") for m in range(MC)]
                an = ans.pop((qc, qt))
                for m in range(MC):
                    tp = mx.tile([P, P], F16, tag="mx", name="tpa")
                    nc.tensor.transpose(tp[:], an[:, m * P:(m + 1) * P],
                                        eye_sb[:])
                    nc.vector.tensor_copy(aoT[qc][m][:, qt * P:(qt + 1) * P],
                                          tp[:])
                ob = ob_pool.tile([P, HID], F32, tag="ob", name="ob")
                for half in range(2):
                    c0 = half * F
                    fin = mx.tile([P, F], F32, tag="mx", name="fin")
                    for m in range(MC):
                        nc.tensor.matmul(fin[:],
                                         aoT[qc][m][:, qt * P:(qt + 1) * P],
                                         woT[:, m * HID + c0:m * HID + c0 + F],
                                         start=(m == 0), stop=(m == MC - 1))
                    nc.gpsimd.tensor_copy(ob[:, c0:c0 + F], fin[:])
                s0 = (NQ * qc + qt) * P
                nc.sync.dma_start(out_d[s0:s0 + P, :], ob[:])

            # ---- emission schedule ----
            fills = deque()
            for s in range(4, SC):
                fills.append(("A", s))
                fills.append(("B", s))

            def pop_fill():
                if not fills:
                    return
                kind, a = fills.popleft()
                if kind == "A":
                    pvb[a] = emit_projA(a)
                elif kind == "B":
                    emit_projB(pvb.pop(a))
                else:
                    emit_oproj(*a)

            for s in range(4):
                st = emit_projA(s)
                emit_projB(st)
            for c in range(1, NQ):
                nc.sync.dma_start(xTp[:, c * (XW // 4):(c + 1) * (XW // 4)],
                                  xTp_d[:, c * (XW // 4):(c + 1) * (XW // 4)])
                nc.sync.dma_start(csn[:, c * (CW // 4):(c + 1) * (CW // 4)],
                                  csn_d[:, c * (CW // 4):(c + 1) * (CW // 4)])

            for qc in range(NQ):
                exs.clear()
                last = NQ * qc + 3
                for kc in range(last + 1):
                    pop_fill()
                    if qc > 0:
                        pop_fill()
                    for m in range(MC):
                        emit_scores(qc, kc, m)
                    if kc >= NQ * qc:
                        qt = kc - NQ * qc
                        po = emit_pv(qc, qt)
                        emit_norm(qc, qt, po)
                        fills.append(("O", (qc, qt)))
            while fills:
                pop_fill()
    nc.compile()
    return nc


def _rope_tables():
    inv_freq = 1.0 / (ROPE_THETA ** (np.arange(0, D, 2, dtype=np.float32) / D))
    t = np.arange(S, dtype=np.float32)
    freqs = np.outer(t, inv_freq)                       # [S, 32]
    emb = np.concatenate([freqs, freqs], axis=-1)       # [S, 64]
    cos = np.cos(emb).astype(np.float32)
    sin = np.sin(emb).astype(np.float32)
    sin_signed = sin.copy()
    sin_signed[:, 0:32] *= -1.0                         # fold rotate_half sign
    cos6 = np.tile(cos, (1, NH))                        # [S, 384]
    sin6 = np.tile(sin_signed, (1, NH))
    # pack [cos | sin] per s-tile: [128, 16*768]
    both = np.concatenate(
        [cos6.reshape(SC, P, F), sin6.reshape(SC, P, F)], axis=2)  # [16,128,768]
    return np.ascontiguousarray(
        both.transpose(1, 0, 2).reshape(P, CW)).astype(np.float16)


_STATE = {}


def _get_program():
    if "nc" not in _STATE:
        _STATE["nc"] = build_program()
    return _STATE["nc"]


def _pack_x(xT):
    """[768, 2048] -> [128, 12288] with cols (c, f, 512)."""
    v = xT.reshape(FC, P, NQ, QW)               # f, p, c, col
    return np.ascontiguousarray(
        v.transpose(1, 2, 0, 3).reshape(P, XW))  # p, (c f col)


def _pack_w(Wq, Wk, Wv, cols):
    """3x [768, 384] (transposed slices) -> [128, 6912] cols (f, kind, 384)."""
    ws = [np.asarray(W[cols, :].T, dtype=np.float32).reshape(FC, P, F)
          for W in (Wq, Wk, Wv)]
    stk = np.stack(ws, axis=2)                   # f, p, kind, 384
    return np.ascontiguousarray(stk.transpose(1, 0, 2, 3).reshape(P, WW))


def _make_in_maps(hidden_states, Wq, Wk, Wv, Wo):
    hs = np.asarray(hidden_states, dtype=np.float32)
    Wq = np.asarray(Wq, dtype=np.float32)
    Wk = np.asarray(Wk, dtype=np.float32)
    Wv = np.asarray(Wv, dtype=np.float32)
    Wo = np.asarray(Wo, dtype=np.float32)

    csn = _rope_tables()
    trineg = (NEG * np.tril(np.ones((P, P), dtype=np.float32), -1)
              ).astype(np.float16)
    eye = np.eye(P, dtype=np.float16)

    in_maps = []
    for c in range(N_CORES):
        b, g = c // 2, c % 2
        cols = slice(g * F, (g + 1) * F)
        woT = np.asarray(Wo[:, cols].T, dtype=np.float32)    # [384, 768]
        woTp = np.ascontiguousarray(
            woT.reshape(MC, P, HID).transpose(1, 0, 2).reshape(P, MC * HID))
        in_maps.append({
            "xTp": _pack_x(hs[b].T).astype(np.float16),
            "wp": _pack_w(Wq, Wk, Wv, cols).astype(np.float16),
            "woT": woTp.astype(np.float16),
            "csn": csn,
            "trineg": trineg,
            "eye": eye,
        })
    return in_maps


def run(hidden_states, Wq, Wk, Wv, Wo, trace=False, **trace_kw):
    nc = _get_program()
    in_maps = _make_in_maps(hidden_states, Wq, Wk, Wv, Wo)
    res = run_bass_kernel_spmd(nc, in_maps, core_ids=list(range(N_CORES)),
                               trace=trace, **trace_kw)
    B = 4
    out = np.empty((B, S, HID), dtype=np.float32)
    for b in range(B):
        out[b] = res.results[2 * b]["out"] + res.results[2 * b + 1]["out"]
    return out, res


def kernel(hidden_states, Wq, Wk, Wv, Wo):
    out, _ = run(hidden_states, Wq, Wk, Wv, Wo,
                 trace=bool(int(os.environ.get("KERNEL_TRACE", "0"))))
    return out


# revision 13
# speedup vs baseline: 1.4661x; 1.0502x over previous
"""Trainium2 Bass kernel for HNet attention (B=4, S=2048, H=768, 12 heads, RoPE, causal).

Sharding: 8 cores = 4 batches x 2 head-groups (6 heads each).
Wq/Wk/Wv split column-wise (head axis), Wo row-wise; host sums the two
partial o_proj outputs per batch (the "all-reduce" done at gather time).

Per-core dataflow (v4 — fp16 inputs, natural-layout PV, packed DMAs):
  xT [768,2048] fp16 (host-packed by column-chunk) --PE--> Q,K,V natural
  RoPE on Q,K in natural layout (DVE t1 / Pool t2), PE-transpose fp16
  scoresT[k,q] = kT.T @ qT per (head-pair, par) with PE row groups; causal
    mask folded into the PE as an accumulate-matmul of a -30000 triangle
  exp on ScalarE -> ex fp16; PV natural: po[q, 65] += ex.T @ [V_h | 1]
  (col 64 = softmax sums); normalize via reciprocal + per-head tensor_scalar;
  deferred fill: PE-transpose -> aoT, o_proj fin = aoT.T @ woT, strip store.
"""

import os
import sys

import numpy as np

sys.path.insert(0, "/opt/trn_rl_repo")

from collections import deque
from contextlib import ExitStack

import concourse.bacc as bacc
import concourse.tile as tile
from concourse import mybir
from concourse.bass_utils import run_bass_kernel_spmd

S = 2048
HID = 768
NH = 6            # heads per core
D = 64
F = NH * D        # 384 per-core feature slice
P = 128
SC = S // P       # 16 s-tiles
FC = HID // P     # 6 contraction chunks
MC = F // P       # 3 head-pair chunks
QW = 512          # q strip width
NQ = S // QW      # 4 strips
N_CORES = 8
ROPE_THETA = 10000.0
NEG = -30000.0

F32 = mybir.dt.float32
F16 = mybir.dt.float16
F8 = mybir.dt.float8e4
AF = mybir.ActivationFunctionType

XW = NQ * FC * QW      # packed xT width 12288
WW = FC * 3 * F        # packed wqkv width 6912
CW = SC * 2 * F        # packed cos|sin width 12288


def build_program():
    nc = bacc.Bacc("TRN2", target_bir_lowering=False, debug=False,
                   num_devices=N_CORES)

    xTp_d = nc.dram_tensor("xTp", [P, XW], F16, kind="ExternalInput").ap()
    wp_d = nc.dram_tensor("wp", [P, WW], F16, kind="ExternalInput").ap()
    woT_d = nc.dram_tensor("woT", [P, MC * HID], F16, kind="ExternalInput").ap()
    csn_d = nc.dram_tensor("csn", [P, CW], F16, kind="ExternalInput").ap()
    tri_d = nc.dram_tensor("trineg", [P, P], F16, kind="ExternalInput").ap()
    eye_d = nc.dram_tensor("eye", [P, P], F16, kind="ExternalInput").ap()
    out_d = nc.dram_tensor("out", [S, HID], F32, kind="ExternalOutput").ap()

    with tile.TileContext(nc) as tc, ExitStack() as ctx:
        const_pool = ctx.enter_context(tc.tile_pool(name="const", bufs=1))
        eye_sb = const_pool.tile([P, P], F16, tag="eye")
        nc.sync.dma_start(eye_sb[:], eye_d[:])
        tri_sb = const_pool.tile([P, P], F16, tag="tri")
        nc.sync.dma_start(tri_sb[:], tri_d[:])

        # ---- persistent SBUF; DMA order feeds the prologue first ----
        xw_pool = ctx.enter_context(tc.tile_pool(name="xw", bufs=1))
        wp = xw_pool.tile([P, WW], F16, tag="wp")
        xTp = xw_pool.tile([P, XW], F16, tag="xTp")
        csn = xw_pool.tile([P, CW], F16, tag="csn")
        woT = xw_pool.tile([P, MC * HID], F16, tag="woT")
        nc.sync.dma_start(wp[:, 0:WW // 2], wp_d[:, 0:WW // 2])
        nc.sync.dma_start(wp[:, WW // 2:WW], wp_d[:, WW // 2:WW])
        nc.sync.dma_start(xTp[:, 0:XW // 4], xTp_d[:, 0:XW // 4])
        nc.sync.dma_start(csn[:, 0:CW // 4], csn_d[:, 0:CW // 4])
        nc.sync.dma_start(woT[:], woT_d[:])

        def wslice(kind, f):  # 0=q 1=k 2=v
            c0 = f * 3 * F + kind * F
            return wp[:, c0:c0 + F]

        def xslice(f, s):
            c0 = (s // 4) * (FC * QW) + f * QW + (s % 4) * P
            return xTp[:, c0:c0 + P]

        kT_pool = ctx.enter_context(tc.tile_pool(name="kTp", bufs=1))
        kTs = kT_pool.tile([P, MC * S], F16, tag="kTs")
        v_pool = ctx.enter_context(tc.tile_pool(name="vp", bufs=1))
        vo = [v_pool.tile([P, NH * 65], F16, tag=f"v{s}", name=f"v{s}")
              for s in range(SC)]
        for s in range(SC):
            v3 = vo[s].rearrange("p (h e) -> p h e", h=NH)
            nc.gpsimd.memset(v3[:, :, 64], 1.0)

        with tc.tile_pool(name="rp", bufs=2) as rp_pool, \
             tc.tile_pool(name="qr", bufs=4) as qr_pool, \
             tc.tile_pool(name="qTs", bufs=2) as qTs_pool, \
             tc.tile_pool(name="ao", bufs=3) as ao_pool, \
             tc.tile_pool(name="ex", bufs=54) as ex_pool, \
             tc.tile_pool(name="an", bufs=3) as an_pool, \
             tc.tile_pool(name="iv", bufs=4) as iv_pool, \
             tc.tile_pool(name="ob", bufs=2) as ob_pool, \
             tc.tile_pool(name="mx", bufs=2, space="PSUM") as mx, \
             tc.tile_pool(name="sc", bufs=2, space="PSUM") as scp, \
             tc.tile_pool(name="po", bufs=2, space="PSUM") as pop:

            qTs = {}   # strip qc -> [3 tiles [P, QW] f16]
            aoT = {}   # strip qc -> [3 tiles [P, QW] f16]
            ans = {}   # (qc, qt) -> normalized ao_nat tile
            exs = {}   # (kc, m) -> (ex0, ex1)
            obs = {}   # strip qc -> packed output staging tile
            pvb = {}   # live projB state per s

            def rope(pp, s):
                """psum natural QKV chunk [P, F] -> rotated fp16 sbuf tile.
                t1 = x*cos on DVE; t2 = rot_half(x)*sin_signed on Pool."""
                cs = csn[:, s * 2 * F:s * 2 * F + F]
                sn = csn[:, s * 2 * F + F:s * 2 * F + 2 * F]
                p3 = pp.rearrange("p (h d) -> p h d", h=NH)
                s3 = sn.rearrange("p (h d) -> p h d", h=NH)
                t1 = rp_pool.tile([P, F], F32, tag="t1", name="t1")
                nc.vector.tensor_mul(t1[:], pp[:], cs[:])
                t2 = rp_pool.tile([P, F], F32, tag="t2", name="t2")
                t23 = t2.rearrange("p (h d) -> p h d", h=NH)
                nc.vector.tensor_mul(t23[:, :, 0:32], p3[:, :, 32:64],
                                     s3[:, :, 0:32])
                nc.vector.tensor_mul(t23[:, :, 32:64], p3[:, :, 0:32],
                                     s3[:, :, 32:64])
                qr = qr_pool.tile([P, F], F16, tag="qr", name="qr")
                nc.gpsimd.tensor_add(qr[:], t1[:], t2[:])
                return qr

            def emit_projA(s):
                qc = s // NQ
                if s % 4 == 0:
                    qTs[qc] = qTs_pool.tile([P, MC * QW], F16, tag="qTs",
                                            name="qTs")
                pq = mx.tile([P, F], F32, tag="mx", name="pq")
                for f in range(FC):
                    nc.tensor.matmul(pq[:], xslice(f, s), wslice(0, f),
                                     start=(f == 0), stop=(f == FC - 1))
                pk = mx.tile([P, F], F32, tag="mx", name="pk")
                for f in range(FC):
                    nc.tensor.matmul(pk[:], xslice(f, s), wslice(1, f),
                                     start=(f == 0), stop=(f == FC - 1))
                qr = rope(pq, s)
                return (s, pk, qr)

            def emit_projB(state):
                s, pk, qr = state
                qc, scol = s // NQ, (s % 4) * P
                pv_ = mx.tile([P, F], F32, tag="mx", name="pv")
                for f in range(FC):
                    nc.tensor.matmul(pv_[:], xslice(f, s), wslice(2, f),
                                     start=(f == 0), stop=(f == FC - 1))
                kr = rope(pk, s)
                tq = mx.tile([P, F], F16, tag="mx", name="tpq")
                for m in range(MC):
                    nc.tensor.transpose(tq[:, m * P:(m + 1) * P],
                                        qr[:, m * P:(m + 1) * P], eye_sb[:])
                qd = qTs[qc].rearrange("p (m c) -> p m c", m=MC)
                nc.vector.tensor_copy(qd[:, :, scol:scol + P],
                                      tq.rearrange("p (m c) -> p m c", m=MC))
                tk = mx.tile([P, F], F16, tag="mx", name="tpk")
                for m in range(MC):
                    nc.tensor.transpose(tk[:, m * P:(m + 1) * P],
                                        kr[:, m * P:(m + 1) * P], eye_sb[:])
                kd = kTs.rearrange("p (m c) -> p m c", m=MC)
                nc.vector.tensor_copy(kd[:, :, s * P:(s + 1) * P],
                                      tk.rearrange("p (m c) -> p m c", m=MC))
                v3 = vo[s].rearrange("p (h e) -> p h e", h=NH)
                p3 = pv_.rearrange("p (h d) -> p h d", h=NH)
                nc.vector.tensor_copy(v3[:, :, 0:64], p3[:])

            def emit_scores(qc, kc, m):
                """head-pair m scores for block kc of strip qc + merged exp."""
                q0, k0 = qc * QW, kc * P
                off = max(0, k0 - q0)
                sp = scp.tile([P, 2 * QW], F32, tag="sc", name="sp")
                for par in range(2):
                    b = par * QW
                    d0 = 64 * par
                    lhsT = kTs[d0:d0 + 64, m * S + k0:m * S + k0 + P]
                    rhs = qTs[qc][:, m * QW:(m + 1) * QW]
                    if k0 >= q0:  # diagonal block: fold mask into PE
                        nc.tensor.matmul(sp[:, b + off:b + off + P],
                                         lhsT, rhs[d0:d0 + 64, off:off + P],
                                         start=True, stop=False)
                        nc.tensor.matmul(sp[:, b + off:b + off + P],
                                         eye_sb[:], tri_sb[:],
                                         start=False, stop=True)
                        if off + P < QW:
                            nc.tensor.matmul(sp[:, b + off + P:b + QW],
                                             lhsT, rhs[d0:d0 + 64, off + P:QW],
                                             start=True, stop=True)
                    else:
                        nc.tensor.matmul(sp[:, b:b + QW],
                                         lhsT, rhs[d0:d0 + 64, :],
                                         start=True, stop=True)
                ex = ex_pool.tile([P, 2 * QW], F8, tag="ex", name="ex")
                sp3 = sp.rearrange("p (r c) -> p r c", r=2)
                ex3 = ex.rearrange("p (r c) -> p r c", r=2)
                nc.scalar.activation(ex3[:, :, off:QW], sp3[:, :, off:QW],
                                     AF.Exp, scale=0.125)
                exs[(kc, m)] = ex

            def emit_pv(qc, qt):
                """qt-serial PV: accumulate po over all kc of the strip."""
                t = NQ * qc + qt
                po = pop.tile([P, NH * 65], F32, tag="po", name="po")
                for kc in range(t + 1):
                    for m in range(MC):
                        ex = exs[(kc, m)]
                        for par in range(2):
                            h = 2 * m + par
                            lhsT = ex[:, par * QW + qt * P:
                                      par * QW + (qt + 1) * P]
                            nc.tensor.matmul(po[:, h * 65:h * 65 + 65],
                                             lhsT, vo[kc][:, h * 65:h * 65 + 65],
                                             start=(kc == 0), stop=(kc == t))
                return po

            def emit_norm(qc, qt, po):
                """copy po to SBUF, reciprocal, per-head scale on Pool."""
                pz = iv_pool.tile([P, NH * 65], F32, tag="pz", name="pz")
                nc.vector.tensor_copy(pz[:], po[:])
                pz3 = pz.rearrange("p (h e) -> p h e", h=NH)
                inv = iv_pool.tile([P, NH], F32, tag="inv", name="inv")
                with nc.allow_low_precision(reason="softmax sums"):
                    nc.vector.reciprocal(inv[:], pz3[:, :, 64])
                an = an_pool.tile([P, F], F16, tag="an", name="an")
                for h in range(NH):
                    nc.gpsimd.tensor_scalar_mul(an[:, h * D:(h + 1) * D],
                                                pz3[:, h, 0:D],
                                                inv[:, h:h + 1])
                ans[(qc, qt)] = an

            def emit_oproj(qc, qt):
                """deferred PE fill: transpose ao_nat -> aoT, fin, store."""
                if qt == 0:
                    aoT[qc] = ao_pool.tile([P, MC * QW], F16, tag="aoT",
                                           name="aoT")
                an = ans.pop((qc, qt))
                ta = mx.tile([P, F], F16, tag="mx", name="tpa")
                for m in range(MC):
                    nc.tensor.transpose(ta[:, m * P:(m + 1) * P],
                                        an[:, m * P:(m + 1) * P], eye_sb[:])
                ad = aoT[qc].rearrange("p (m c) -> p m c", m=MC)
                nc.vector.tensor_copy(ad[:, :, qt * P:(qt + 1) * P],
                                      ta.rearrange("p (m c) -> p m c", m=MC))
                ob = ob_pool.tile([P, HID], F32, tag="ob", name="ob")
                for half in range(2):
                    c0 = half * F
                    fin = mx.tile([P, F], F32, tag="mx", name="fin")
                    for m in range(MC):
                        nc.tensor.matmul(fin[:],
                                         aoT[qc][:, m * QW + qt * P:
                                                 m * QW + (qt + 1) * P],
                                         woT[:, m * HID + c0:m * HID + c0 + F],
                                         start=(m == 0), stop=(m == MC - 1))
                    nc.vector.tensor_copy(ob[:, c0:c0 + F], fin[:])
                s0 = (NQ * qc + qt) * P
                nc.sync.dma_start(out_d[s0:s0 + P, :], ob[:])

            # ---- emission schedule ----
            fills = deque()
            for s in range(4, SC):
                fills.append(("A", s))
                fills.append(("B", s))

            def pop_fill():
                if not fills:
                    return
                kind, a = fills.popleft()
                if kind == "A":
                    pvb[a] = emit_projA(a)
                elif kind == "B":
                    emit_projB(pvb.pop(a))
                else:
                    emit_oproj(*a)

            for s in range(4):
                st = emit_projA(s)
                emit_projB(st)
            for c in range(1, NQ):
                nc.sync.dma_start(xTp[:, c * (XW // 4):(c + 1) * (XW // 4)],
                                  xTp_d[:, c * (XW // 4):(c + 1) * (XW // 4)])
                nc.sync.dma_start(csn[:, c * (CW // 4):(c + 1) * (CW // 4)],
                                  csn_d[:, c * (CW // 4):(c + 1) * (CW // 4)])

            for qc in range(NQ):
                exs.clear()
                last = NQ * qc + 3
                for kc in range(last + 1):
                    pop_fill()
                    if qc > 0:
                        pop_fill()
                    for m in range(MC):
                        emit_scores(qc, kc, m)
                    if kc >= NQ * qc:
                        qt = kc - NQ * qc
                        po = emit_pv(qc, qt)
                        emit_norm(qc, qt, po)
                        fills.append(("O", (qc, qt)))
            while fills:
                pop_fill()
    nc.compile()
    return nc


def _rope_tables():
    inv_freq = 1.0 / (ROPE_THETA ** (np.arange(0, D, 2, dtype=np.float32) / D))
    t = np.arange(S, dtype=np.float32)
    freqs = np.outer(t, inv_freq)                       # [S, 32]
    emb = np.concatenate([freqs, freqs], axis=-1)       # [S, 64]
    cos = np.cos(emb).astype(np.float32)
    sin = np.sin(emb).astype(np.float32)
    sin_signed = sin.copy()
    sin_signed[:, 0:32] *= -1.0                         # fold rotate_half sign
    cos6 = np.tile(cos, (1, NH))                        # [S, 384]
    sin6 = np.tile(sin_signed, (1, NH))
    # pack [cos | sin] per s-tile: [128, 16*768]
    both = np.concatenate(
        [cos6.reshape(SC, P, F), sin6.reshape(SC, P, F)], axis=2)  # [16,128,768]
    return np.ascontiguousarray(
        both.transpose(1, 0, 2).reshape(P, CW)).astype(np.float16)


_STATE = {}


def _get_program():
    if "nc" not in _STATE:
        _STATE["nc"] = build_program()
    return _STATE["nc"]


def _pack_x(xT):
    """[768, 2048] -> [128, 12288] with cols (c, f, 512)."""
    v = xT.reshape(FC, P, NQ, QW)               # f, p, c, col
    return np.ascontiguousarray(
        v.transpose(1, 2, 0, 3).reshape(P, XW))  # p, (c f col)


def _pack_w(Wq, Wk, Wv, cols):
    """3x [768, 384] (transposed slices) -> [128, 6912] cols (f, kind, 384)."""
    ws = [np.asarray(W[cols, :].T, dtype=np.float32).reshape(FC, P, F)
          for W in (Wq, Wk, Wv)]
    stk = np.stack(ws, axis=2)                   # f, p, kind, 384
    return np.ascontiguousarray(stk.transpose(1, 0, 2, 3).reshape(P, WW))


def _make_in_maps(hidden_states, Wq, Wk, Wv, Wo):
    hs = np.asarray(hidden_states, dtype=np.float32)
    Wq = np.asarray(Wq, dtype=np.float32)
    Wk = np.asarray(Wk, dtype=np.float32)
    Wv = np.asarray(Wv, dtype=np.float32)
    Wo = np.asarray(Wo, dtype=np.float32)

    csn = _rope_tables()
    trineg = (NEG * np.tril(np.ones((P, P), dtype=np.float32), -1)
              ).astype(np.float16)
    eye = np.eye(P, dtype=np.float16)

    in_maps = []
    for c in range(N_CORES):
        b, g = c // 2, c % 2
        cols = slice(g * F, (g + 1) * F)
        woT = np.asarray(Wo[:, cols].T, dtype=np.float32)    # [384, 768]
        woTp = np.ascontiguousarray(
            woT.reshape(MC, P, HID).transpose(1, 0, 2).reshape(P, MC * HID))
        in_maps.append({
            "xTp": _pack_x(hs[b].T).astype(np.float16),
            "wp": _pack_w(Wq, Wk, Wv, cols).astype(np.float16),
            "woT": woTp.astype(np.float16),
            "csn": csn,
            "trineg": trineg,
            "eye": eye,
        })
    return in_maps


def run(hidden_states, Wq, Wk, Wv, Wo, trace=False, **trace_kw):
    nc = _get_program()
    in_maps = _make_in_maps(hidden_states, Wq, Wk, Wv, Wo)
    res = run_bass_kernel_spmd(nc, in_maps, core_ids=list(range(N_CORES)),
                               trace=trace, **trace_kw)
    B = 4
    out = np.empty((B, S, HID), dtype=np.float32)
    for b in range(B):
        out[b] = res.results[2 * b]["out"] + res.results[2 * b + 1]["out"]
    return out, res


def kernel(hidden_states, Wq, Wk, Wv, Wo):
    out, _ = run(hidden_states, Wq, Wk, Wv, Wo,
                 trace=bool(int(os.environ.get("KERNEL_TRACE", "0"))))
    return out
